# revision 30
# baseline (speedup 1.0000x reference)
"""Trainium2 Bass kernel for nn_BasicTransformerBlock_50208167690869.

Sparse-attention transformer block, sharded data-parallel over the 16-frame
axis across 8 NeuronCores (2 frames/core, 1-frame halo for the per-frame
shift). The cross-frame cumsum of the compression branch is realized with an
on-device AllGather of each core's local comp sum plus a masked prefix
reduction. Top-k selection is computed on host with the exact same jax ops
as the reference (bitwise-identical selection; the ranking is
rounding-critical), everything else runs on device.

Self-contained: hardcodes shapes from the problem spec.
"""
import sys

for _p in ("/opt/trn_rl_repo",):
    if _p not in sys.path:
        sys.path.append(_p)

import numpy as np
import ml_dtypes

HEADS = 8
DH = 80
D = 640
F = 16
S = 1024
K = 512
NCORES = 8
FPC = F // NCORES  # frames per core
SCALE = DH ** -0.5
NKC = D // 128     # 5 contraction chunks
NTC = S // 128     # 8 token chunks
NSC = K // 128     # 4 slot chunks

_cache = {}


def _apply_drain_patch():
    """This walrus build rejects >1 sync-wait on CTRL instructions; split the
    TileContext tail-drain waits across single-wait nops."""
    import concourse.tile as tile
    from concourse.vector_clock import ScopedClock, VectorClock

    if getattr(tile.TileContext, "_drain_patched", False):
        return

    def _patched(self, tick_clock, wait_clock):
        nc = self.nc
        gc = tick_clock.global_clock
        n = len(gc)
        for p in range(n):
            t = gc[p]
            if t == 0:
                continue
            vc = VectorClock([t if i == p else 0 for i in range(n)])
            nop_inst = nc.sync.nop()
            wait_clock.add_sem_waits(nop_inst.ins, ScopedClock({None: vc}))
        nc.sync.drain()
        nc.all_engine_barrier()
        assert self.sems is not None
        popped = nc._tile_sem_poison_stack.pop()
        assert popped is self._sem_poison
        nc.clear_and_free_semaphores(list(self.sems.allocated().values()))
        nc.all_engine_barrier()

    tile.TileContext._drain_and_barrier = _patched
    tile.TileContext._drain_patched = True


def _build_nc():
    import concourse.bass as bass
    import concourse.bacc as bacc
    import concourse.mybir as mybir
    import concourse.tile as tile
    from concourse.masks import make_identity

    _apply_drain_patch()

    bf16 = mybir.dt.bfloat16
    f32 = mybir.dt.float32
    i32 = mybir.dt.int32
    ADD = mybir.AluOpType.add
    SUB = mybir.AluOpType.subtract
    MUL = mybir.AluOpType.mult

    nc = bacc.Bacc("TRN2", target_bir_lowering=False, num_devices=NCORES)

    # ---- params ----
    hsT = nc.declare_dram_parameter("hsT", [3, D, S], bf16, isOutput=False)
    g1t = nc.declare_dram_parameter("g1t", [FPC, D, K], bf16, isOutput=False)
    g2t = nc.declare_dram_parameter("g2t", [FPC, D, K], bf16, isOutput=False)
    topk = nc.declare_dram_parameter("topk", [FPC, K], i32, isOutput=False)
    wnames = ["wq", "wk", "wv", "wo", "wcq", "wck", "wcv", "wco", "wprim"]
    wd = {n: nc.declare_dram_parameter(n, [D, D], bf16, isOutput=False) for n in wnames}
    b_prim = nc.declare_dram_parameter("b_prim", [D], f32, isOutput=False)
    b_cq = nc.declare_dram_parameter("b_cq", [D], f32, isOutput=False)
    b_ck = nc.declare_dram_parameter("b_ck", [D], f32, isOutput=False)
    brep_cv = nc.declare_dram_parameter("brep_cv", [128, D], bf16, isOutput=False)
    brep_co = nc.declare_dram_parameter("brep_co", [128, D], bf16, isOutput=False)
    brep_o = nc.declare_dram_parameter("brep_o", [128, D], bf16, isOutput=False)
    wmask = nc.declare_dram_parameter("wmask", [128, NCORES], f32, isOutput=False)
    out_p = nc.declare_dram_parameter("out", [FPC, S, D], f32, isOutput=True)

    with tile.TileContext(nc) as tc:
        with tc.tile_pool(name="wp", bufs=1) as wp, \
             tc.tile_pool(name="pfm", bufs=1) as pfm, \
             tc.tile_pool(name="small", bufs=2) as small, \
             tc.tile_pool(name="pexp", bufs=4) as pexp, \
             tc.tile_pool(name="psA", bufs=2, space="PSUM") as psA, \
             tc.tile_pool(name="psJ", bufs=2, space="PSUM") as psJ, \
             tc.tile_pool(name="psO", bufs=3, space="PSUM") as psO, \
             tc.tile_pool(name="psT", bufs=1, space="PSUM") as psT, \
             tc.tile_pool(name="dram", bufs=1, space="DRAM") as dram:

            # ---- persistent constants ----
            ident = wp.tile([128, 128], bf16, tag="ident")
            make_identity(nc, ident[:])
            ones_row = wp.tile([1, 128], bf16, tag="ones")
            nc.gpsimd.memset(ones_row[:], 1.0)

            bprim_sb = wp.tile([128, NKC], f32, tag="bprim")
            for kc in range(NKC):
                nc.sync.dma_start(out=bprim_sb[:, kc:kc + 1], in_=b_prim[kc * 128:(kc + 1) * 128, None])
            bcq_sb = wp.tile([DH, HEADS], f32, tag="bcq")
            bck_sb = wp.tile([DH, HEADS], f32, tag="bck")
            for h in range(HEADS):
                nc.sync.dma_start(out=bcq_sb[:, h:h + 1], in_=b_cq[DH * h:DH * (h + 1), None])
                nc.sync.dma_start(out=bck_sb[:, h:h + 1], in_=b_ck[DH * h:DH * (h + 1), None])
            brepcv_sb = wp.tile([128, D], bf16, tag="brepcv")
            nc.sync.dma_start(out=brepcv_sb[:], in_=brep_cv[:])
            brepco_sb = wp.tile([128, D], bf16, tag="brepco")
            nc.sync.dma_start(out=brepco_sb[:], in_=brep_co[:])
            brepo_sb = wp.tile([128, D], bf16, tag="brepo")
            nc.sync.dma_start(out=brepo_sb[:], in_=brep_o[:])
            wmask_sb = wp.tile([128, NCORES], f32, tag="wmask")
            nc.sync.dma_start(out=wmask_sb[:], in_=wmask[:])

            wsb = {}
            _weng = [nc.scalar, nc.sync, nc.gpsimd]
            for wi, n in enumerate(["wprim", "wcq", "wck", "wcv", "wq", "wk", "wv"]):
                t = wp.tile([128, NKC, D], bf16, tag=f"w_{n}")
                for kc in range(NKC):
                    _weng[(wi + kc) % 3].dma_start(out=t[:, kc, :], in_=wd[n][kc * 128:(kc + 1) * 128, :])
                wsb[n] = t
            for wi, n in enumerate(["wco", "wo"]):
                t = wp.tile([DH, HEADS, D], bf16, tag=f"w_{n}")
                for h in range(HEADS):
                    _weng[(wi + h) % 3].dma_start(out=t[:, h, :], in_=wd[n][DH * h:DH * (h + 1), :])
                wsb[n] = t

            def load_hs(slot, nm):
                t = pfm.tile([128, NKC, S], bf16, tag="hs", name=nm, bufs=2)
                _he = [nc.scalar, nc.sync]
                for kc in range(NKC):
                    _he[kc % 2].dma_start(out=t[:, kc, :], in_=hsT[slot, kc * 128:(kc + 1) * 128, :])
                return t

            # ---- DRAM scratch ----
            compd = dram.tile([FPC, K, D], f32, tag="compd")
            cc_in = dram.tile([K, D], f32, tag="cc_in")
            cc_out = dram.tile([NCORES, K, D], f32, tag="cc_out")

            # ---- helpers ----
            def proj_fm_head(w, bhead, src, N, out_tag):
                """Per-head FM projection: out [DH, HEADS, N] bf16 = W.T @ src (+bias)."""
                o = pfm.tile([DH, HEADS, N], bf16, tag=out_tag)
                for h in range(HEADS):
                    for n0 in range(0, N, 512):
                        nn = min(512, N - n0)
                        ps = psJ.tile([128, 512], f32, tag="psJ")
                        for kc in range(NKC):
                            nc.tensor.matmul(
                                out=ps[0:DH, 0:nn],
                                lhsT=w[:, kc, DH * h:DH * (h + 1)],
                                rhs=src[:, kc, n0:n0 + nn],
                                start=(kc == 0), stop=(kc == NKC - 1))
                        if bhead is not None:
                            nc.vector.tensor_scalar(
                                o[:, h, n0:n0 + nn], ps[0:DH, 0:nn],
                                bhead[:, h:h + 1], None, op0=ADD)
                        else:
                            nc.scalar.copy(o[:, h, n0:n0 + nn], ps[0:DH, 0:nn])
                return o

            def proj_tm_v(src, w, brep, frame_tag):
                """TM projection with ones column: list of 8 tiles [128, HEADS, DH+1]."""
                tiles = []
                for tcid in range(NTC):
                    psL = psJ.tile([128, 512], f32, tag="psJ")
                    psR = psJ.tile([128, 512], f32, tag="psJ")
                    for kc in range(NKC):
                        nc.tensor.matmul(
                            out=psL[:, 0:512],
                            lhsT=src[:, kc, tcid * 128:(tcid + 1) * 128],
                            rhs=w[:, kc, 0:512],
                            start=(kc == 0), stop=(kc == NKC - 1))
                    for kc in range(NKC):
                        nc.tensor.matmul(
                            out=psR[:, 0:128],
                            lhsT=src[:, kc, tcid * 128:(tcid + 1) * 128],
                            rhs=w[:, kc, 512:640],
                            start=(kc == 0), stop=(kc == NKC - 1))
                    v = pfm.tile([128, HEADS, 97], bf16, tag=f"{frame_tag}{tcid}")
                    nc.vector.memset(v[:, :, DH:96], 0.0)
                    nc.vector.memset(v[:, :, 96:97], 1.0)

                    def _vcopy(hh, d0, d1, srcap, c0):
                        if brep is not None:
                            nc.vector.tensor_tensor(
                                out=v[:, hh, d0:d1], in0=srcap,
                                in1=brep[:, c0:c0 + (d1 - d0)], op=ADD)
                        else:
                            nc.vector.tensor_copy(v[:, hh, d0:d1], srcap)

                    for h in range(6):
                        _vcopy(h, 0, DH, psL[:, DH * h:DH * (h + 1)], DH * h)
                    _vcopy(6, 0, 32, psL[:, 480:512], 480)
                    _vcopy(6, 32, DH, psR[:, 0:48], 512)
                    _vcopy(7, 0, DH, psR[:, 48:128], 560)
                    tiles.append(v)
                return tiles

            def attention(qT, kT, vts, NQ, out_tag):
                """FM attention: O^T [DH, HEADS, NQ] bf16, softmax over S keys."""
                o = pfm.tile([DH, HEADS, NQ], bf16, tag=out_tag)
                pending = []

                def flush_norm():
                    if not pending:
                        return
                    po, h, n0, nn = pending.pop(0)
                    drow_bf = small.tile([1, 512], bf16, tag="drowbf")
                    nc.scalar.copy(out=drow_bf[:, 0:nn], in_=po[96:97, 0:nn])
                    prep = psA.tile([128, 512], f32, tag="psA")
                    nc.tensor.matmul(
                        out=prep[:, 0:nn], lhsT=ones_row[:],
                        rhs=drow_bf[:, 0:nn], start=True, stop=True)
                    recip_sb = pexp.tile([128, 512], f32, tag="recipsb", bufs=2)
                    nc.vector.reciprocal(out=recip_sb[0:DH, 0:nn], in_=prep[0:DH, 0:nn])
                    nc.vector.tensor_tensor(
                        out=o[:, h, n0:n0 + nn], in0=po[0:DH, 0:nn],
                        in1=recip_sb[0:DH, 0:nn], op=MUL)

                for h in range(HEADS):
                    for n0 in range(0, NQ, 512):
                        nn = min(512, NQ - n0)
                        po = psO.tile([128, 512], f32, tag="psO")
                        for tcid in range(NTC):
                            ps = psA.tile([128, 512], f32, tag="psA")
                            nc.tensor.matmul(
                                out=ps[:, 0:nn],
                                lhsT=kT[:, h, tcid * 128:(tcid + 1) * 128],
                                rhs=qT[:, h, n0:n0 + nn],
                                start=True, stop=True)
                            ex = pexp.tile([128, 512], bf16, tag="exp", bufs=3)
                            nc.scalar.activation(
                                out=ex[:, 0:nn], in_=ps[:, 0:nn],
                                func=mybir.ActivationFunctionType.Exp)
                            nc.tensor.matmul(
                                out=po[0:97, 0:nn],
                                lhsT=vts[tcid][:, h, 0:97],
                                rhs=ex[:, 0:nn],
                                start=(tcid == 0), stop=(tcid == NTC - 1))
                        pending.append((po, h, n0, nn))
                        if len(pending) > 1:
                            flush_norm()
                while pending:
                    flush_norm()
                return o

            # ================= COMP BRANCH (both frames first) =================
            ssum = [pfm.tile([128, D], f32, tag=f"ssum{sc}", name=f"ssum{sc}") for sc in range(NSC)]
            hs_tiles = []
            for f in range(FPC):
                hs_prev = load_hs(f, f"hsc{f}")
                hs_tiles.append(hs_prev)
                g1 = pfm.tile([128, NKC, K], bf16, tag="g1")
                g2 = pfm.tile([128, NKC, K], bf16, tag="g2")
                _ge = nc.gpsimd if f == 0 else nc.sync
                for kc in range(NKC):
                    _ge.dma_start(out=g1[:, kc, :], in_=g1t[f, kc * 128:(kc + 1) * 128, :])
                    _ge.dma_start(out=g2[:, kc, :], in_=g2t[f, kc * 128:(kc + 1) * 128, :])
                gd = pfm.tile([128, NKC, K], bf16, tag="zt")
                nc.vector.tensor_tensor(out=gd[:], in0=g1[:], in1=g2[:], op=SUB)

                # primary_hs (FM, chunk-aligned): prim = gd @ Wprim + b + g1
                prim = pfm.tile([128, NKC, K], bf16, tag="prim")
                for mc in range(NKC):
                    ps = psJ.tile([128, 512], f32, tag="psJ")
                    for kc in range(NKC):
                        nc.tensor.matmul(
                            out=ps[:], lhsT=wsb["wprim"][:, kc, mc * 128:(mc + 1) * 128],
                            rhs=gd[:, kc, :], start=(kc == 0), stop=(kc == NKC - 1))
                    nc.vector.tensor_scalar(
                        prim[:, mc, :], ps[:], bprim_sb[:, mc:mc + 1], None, op0=ADD)
                    nc.vector.tensor_tensor(
                        out=prim[:, mc, :], in0=prim[:, mc, :], in1=g1[:, mc, :], op=ADD)

                qcT = proj_fm_head(wsb["wcq"], bcq_sb, prim, K, "qT")
                kcT = proj_fm_head(wsb["wck"], bck_sb, hs_prev, S, "kT")
                vC = proj_tm_v(hs_prev, wsb["wcv"], brepcv_sb, "v")
                ocT = attention(qcT, kcT, vC, K, "oT")

                # comp_out projection (TM) + bias; spill to DRAM + local sum
                for sc in range(NSC):
                    psL = psJ.tile([128, 512], f32, tag="psJ")
                    psR = psJ.tile([128, 512], f32, tag="psJ")
                    for h in range(HEADS):
                        nc.tensor.matmul(
                            out=psL[:, 0:512],
                            lhsT=ocT[:, h, sc * 128:(sc + 1) * 128],
                            rhs=wsb["wco"][:, h, 0:512],
                            start=(h == 0), stop=(h == HEADS - 1))
                    for h in range(HEADS):
                        nc.tensor.matmul(
                            out=psR[:, 0:128],
                            lhsT=ocT[:, h, sc * 128:(sc + 1) * 128],
                            rhs=wsb["wco"][:, h, 512:640],
                            start=(h == 0), stop=(h == HEADS - 1))
                    csb = small.tile([128, D], f32, tag="wk640")
                    nc.vector.tensor_tensor(out=csb[:, 0:512], in0=psL[:, 0:512],
                                            in1=brepco_sb[:, 0:512], op=ADD)
                    nc.vector.tensor_tensor(out=csb[:, 512:640], in0=psR[:, 0:128],
                                            in1=brepco_sb[:, 512:640], op=ADD)
                    nc.sync.dma_start(out=compd[f, sc * 128:(sc + 1) * 128, :], in_=csb[:])
                    if f == 0:
                        nc.vector.tensor_copy(ssum[sc][:], csb[:])
                    else:
                        nc.vector.tensor_tensor(out=ssum[sc][:], in0=ssum[sc][:], in1=csb[:], op=ADD)

            for sc in range(NSC):
                nc.sync.dma_start(out=cc_in[sc * 128:(sc + 1) * 128, :], in_=ssum[sc][:])
            nc.gpsimd.collective_compute(
                "AllGather", mybir.AluOpType.bypass,
                replica_groups=[list(range(NCORES))],
                ins=[cc_in.opt()], outs=[cc_out.opt()])

            # masked prefix, entirely on the gpsimd queue (inherently ordered
            # after the collective; keeps the PE/DVE/sync queues unblocked)
            pfx = [pfm.tile([128, D], f32, tag=f"ssum{sc}", name=f"pfx{sc}")
                   for sc in range(NSC)]
            for sc in range(NSC):
                nc.gpsimd.memset(pfx[sc][:], 0.0)
                for cc in range(NCORES):
                    tmp = small.tile([128, D], f32, tag="wk640b")
                    nc.gpsimd.dma_start(out=tmp[:], in_=cc_out[cc, sc * 128:(sc + 1) * 128, :])
                    nc.gpsimd.tensor_tensor(
                        out=tmp[:], in0=tmp[:],
                        in1=wmask_sb[:, cc:cc + 1].to_broadcast([128, D]), op=MUL)
                    nc.gpsimd.tensor_tensor(out=pfx[sc][:], in0=pfx[sc][:], in1=tmp[:], op=ADD)

            # ================= MAIN BRANCH =================
            for f in range(FPC):
                if f == 0:
                    hs_prev = hs_tiles[0]
                    hs_own = hs_tiles[1]
                else:
                    hs_prev = hs_tiles[1]
                    hs_own = load_hs(2, "hsm2")
                q2T = proj_fm_head(wsb["wq"], None, hs_own, S, "qT")
                k2T = proj_fm_head(wsb["wk"], None, hs_prev, S, "kT")
                v2 = proj_tm_v(hs_prev, wsb["wv"], None, "v")
                o2T = attention(q2T, k2T, v2, S, "oT")

                # out projection -> Z^T bf16
                zt = pfm.tile([128, NKC, S], bf16, tag="zt")
                for mc in range(NKC):
                    for n0 in range(0, S, 512):
                        ps = psJ.tile([128, 512], f32, tag="psJ")
                        for h in range(HEADS):
                            nc.tensor.matmul(
                                out=ps[:],
                                lhsT=wsb["wo"][:, h, mc * 128:(mc + 1) * 128],
                                rhs=o2T[:, h, n0:n0 + 512],
                                start=(h == 0), stop=(h == HEADS - 1))
                        nc.scalar.copy(out=zt[:, mc, n0:n0 + 512], in_=ps[:])

                # transpose to TM, add bias, store
                for tcid in range(NTC):
                    pt = psT.tile([128, D], bf16, tag="psT")
                    for mc in range(NKC):
                        nc.tensor.transpose(
                            out=pt[:, mc * 128:(mc + 1) * 128],
                            in_=zt[:, mc, tcid * 128:(tcid + 1) * 128],
                            identity=ident[:])
                    zfin = small.tile([128, D], f32, tag="wk640")
                    nc.vector.tensor_tensor(out=zfin[:], in0=pt[:], in1=brepo_sb[:], op=ADD)
                    nc.sync.dma_start(out=out_p[f, tcid * 128:(tcid + 1) * 128, :], in_=zfin[:])



            # cumsum + scatter-add RMW, entirely on the gpsimd queue;
            # phased per frame so the 4 indirect gathers pipeline their transfers
            for f in range(FPC):
                opv = out_p[0:1] if f == 0 else out_p[:]
                idxs = []
                for sc in range(NSC):
                    ctile = small.tile([128, D], f32, tag="wk640b")
                    nc.gpsimd.dma_start(out=ctile[:], in_=compd[f, sc * 128:(sc + 1) * 128, :])
                    nc.gpsimd.tensor_tensor(out=pfx[sc][:], in0=pfx[sc][:], in1=ctile[:], op=ADD)
                    idx = small.tile([128, 1], i32, tag="idx8", bufs=8)
                    nc.gpsimd.dma_start(out=idx[:], in_=topk[f, sc * 128:(sc + 1) * 128, None])
                    idxs.append(idx)
                for p0 in range(0, NSC, 2):
                    pair = range(p0, min(p0 + 2, NSC))
                    pg = {}
                    for sc in pair:
                        grow = small.tile([128, D], f32, tag="grow4", bufs=2)
                        nc.gpsimd.indirect_dma_start(
                            out=grow[:], out_offset=None, in_=opv,
                            in_offset=bass.IndirectOffsetOnAxis(ap=idxs[sc][:, :1], axis=1),
                            element_offset=f * S * D)
                        pg[sc] = grow
                    for sc in pair:
                        nc.gpsimd.tensor_tensor(out=pg[sc][:], in0=pg[sc][:], in1=pfx[sc][:], op=ADD)
                    for sc in pair:
                        nc.gpsimd.indirect_dma_start(
                            out=opv, out_offset=bass.IndirectOffsetOnAxis(ap=idxs[sc][:, :1], axis=1),
                            in_=pg[sc][:], in_offset=None,
                            element_offset=f * S * D)

    nc.compile()
    return nc


def _host_prep(hidden_states, topk_idx, weights):
    """Build the 8 per-core input maps (pure data movement + dtype casts)."""
    bf = ml_dtypes.bfloat16
    hs = np.asarray(hidden_states, np.float32)
    (wq, wk, wv, wo, wcq, wck, wcv, wco, wprim,
     b_prim, b_cq, b_ck, b_cv, b_co, b_o) = weights

    wq_s = (wq * SCALE).astype(bf)
    wcq_s = (wcq * SCALE).astype(bf)
    b_cq_s = (b_cq * SCALE).astype(np.float32)
    wcast = {
        "wq": wq_s, "wk": wk.astype(bf), "wv": wv.astype(bf), "wo": wo.astype(bf),
        "wcq": wcq_s, "wck": wck.astype(bf), "wcv": wcv.astype(bf),
        "wco": wco.astype(bf), "wprim": wprim.astype(bf),
    }
    shared = dict(wcast)
    shared["b_prim"] = b_prim.astype(np.float32)
    shared["b_cq"] = b_cq_s
    shared["b_ck"] = b_ck.astype(np.float32)
    shared["brep_cv"] = np.broadcast_to(b_cv.astype(bf), (128, D)).copy()
    shared["brep_co"] = np.broadcast_to(b_co.astype(bf), (128, D)).copy()
    shared["brep_o"] = np.broadcast_to(b_o.astype(bf), (128, D)).copy()

    in_maps = []
    for c in range(NCORES):
        f0, f1 = 2 * c, 2 * c + 1
        p0 = max(f0 - 1, 0)
        m = dict(shared)
        m["hsT"] = np.ascontiguousarray(
            hs[[p0, f0, f1]].transpose(0, 2, 1)).astype(bf)
        g1 = np.stack([hs[f0][topk_idx[f0]], hs[f1][topk_idx[f1]]])
        g2 = np.stack([hs[p0][topk_idx[f0]], hs[f0][topk_idx[f1]]])
        m["g1t"] = np.ascontiguousarray(g1.transpose(0, 2, 1)).astype(bf)
        m["g2t"] = np.ascontiguousarray(g2.transpose(0, 2, 1)).astype(bf)
        m["topk"] = np.ascontiguousarray(topk_idx[[f0, f1]]).astype(np.int32)
        wm = np.zeros((128, NCORES), np.float32)
        wm[:, :c] = 1.0
        m["wmask"] = wm
        in_maps.append(m)
    return in_maps


def kernel(hidden_states, primary_lin_w, primary_lin_b,
           comp_q_w, comp_q_b, comp_k_w, comp_k_b, comp_v_w, comp_v_b,
           comp_out_w, comp_out_b,
           to_q_w, to_k_w, to_v_w, to_out_w, to_out_b,
           video_length):
    import jax
    import jax.numpy as jnp
    from concourse.bass_utils import run_bass_kernel_spmd

    f = int(video_length)
    assert f == F and hidden_states.shape == (F, S, D)

    # Top-k selection: replicate the reference's exact eager-jax computation
    # (selection order is rounding-critical; must match bitwise).
    hs_j = jnp.asarray(np.asarray(hidden_states, np.float32))
    pre = jnp.concatenate([hs_j[:1], hs_j[:-1]], axis=0)
    diff_score = jnp.abs(hs_j - pre).mean(axis=-1)
    _, topk_idx = jax.lax.top_k(diff_score, max(64, S // 2))
    topk_idx = np.asarray(topk_idx)

    weights = tuple(np.asarray(w, np.float32) for w in (
        to_q_w, to_k_w, to_v_w, to_out_w, comp_q_w, comp_k_w, comp_v_w,
        comp_out_w, primary_lin_w, primary_lin_b, comp_q_b, comp_k_b,
        comp_v_b, comp_out_b, to_out_b))
    in_maps = _host_prep(hidden_states, topk_idx, weights)

    if "nc" not in _cache:
        _cache["nc"] = _build_nc()
    res = run_bass_kernel_spmd(_cache["nc"], in_maps, list(range(NCORES)))
    out = np.concatenate([res.results[c]["out"] for c in range(NCORES)], axis=0)
    return np.ascontiguousarray(out.astype(np.float32))


# revision 31
# speedup vs baseline: 1.0795x; 1.0795x over previous
"""Trainium2 Bass kernel for nn_BasicTransformerBlock_50208167690869.

Sparse-attention transformer block, sharded data-parallel over the 16-frame
axis across 8 NeuronCores (2 frames/core, 1-frame halo for the per-frame
shift). The cross-frame cumsum of the compression branch is realized with an
on-device AllGather of each core's local comp sum plus a masked prefix
reduction. Top-k selection is computed on host with the exact same jax ops
as the reference (bitwise-identical selection; the ranking is
rounding-critical), everything else runs on device.

Self-contained: hardcodes shapes from the problem spec.
"""
import sys

for _p in ("/opt/trn_rl_repo",):
    if _p not in sys.path:
        sys.path.append(_p)

import numpy as np
import ml_dtypes

HEADS = 8
DH = 80
D = 640
F = 16
S = 1024
K = 512
NCORES = 8
FPC = F // NCORES  # frames per core
SCALE = DH ** -0.5
NKC = D // 128     # 5 contraction chunks
NTC = S // 128     # 8 token chunks
NSC = K // 128     # 4 slot chunks

_cache = {}


def _apply_drain_patch():
    """This walrus build rejects >1 sync-wait on CTRL instructions; split the
    TileContext tail-drain waits across single-wait nops."""
    import concourse.tile as tile
    from concourse.vector_clock import ScopedClock, VectorClock

    if getattr(tile.TileContext, "_drain_patched", False):
        return

    def _patched(self, tick_clock, wait_clock):
        nc = self.nc
        gc = tick_clock.global_clock
        n = len(gc)
        for p in range(n):
            t = gc[p]
            if t == 0:
                continue
            vc = VectorClock([t if i == p else 0 for i in range(n)])
            nop_inst = nc.sync.nop()
            wait_clock.add_sem_waits(nop_inst.ins, ScopedClock({None: vc}))
        nc.sync.drain()
        nc.all_engine_barrier()
        assert self.sems is not None
        popped = nc._tile_sem_poison_stack.pop()
        assert popped is self._sem_poison
        nc.clear_and_free_semaphores(list(self.sems.allocated().values()))
        nc.all_engine_barrier()

    tile.TileContext._drain_and_barrier = _patched
    tile.TileContext._drain_patched = True


def _build_nc():
    import concourse.bass as bass
    import concourse.bacc as bacc
    import concourse.mybir as mybir
    import concourse.tile as tile
    from concourse.masks import make_identity

    _apply_drain_patch()

    bf16 = mybir.dt.bfloat16
    f32 = mybir.dt.float32
    i32 = mybir.dt.int32
    ADD = mybir.AluOpType.add
    SUB = mybir.AluOpType.subtract
    MUL = mybir.AluOpType.mult

    nc = bacc.Bacc("TRN2", target_bir_lowering=False, num_devices=NCORES)

    # ---- params ----
    hsT = nc.declare_dram_parameter("hsT", [3, D, S], bf16, isOutput=False)
    g1t = nc.declare_dram_parameter("g1t", [FPC, D, K], bf16, isOutput=False)
    g2t = nc.declare_dram_parameter("g2t", [FPC, D, K], bf16, isOutput=False)
    topk = nc.declare_dram_parameter("topk", [FPC, K], i32, isOutput=False)
    wnames = ["wq", "wk", "wv", "wo", "wcq", "wck", "wcv", "wco", "wprim"]
    wd = {n: nc.declare_dram_parameter(n, [D, D], bf16, isOutput=False) for n in wnames}
    b_prim = nc.declare_dram_parameter("b_prim", [D], f32, isOutput=False)
    b_cq = nc.declare_dram_parameter("b_cq", [D], f32, isOutput=False)
    b_ck = nc.declare_dram_parameter("b_ck", [D], f32, isOutput=False)
    brep_cv = nc.declare_dram_parameter("brep_cv", [128, D], bf16, isOutput=False)
    brep_co = nc.declare_dram_parameter("brep_co", [128, D], bf16, isOutput=False)
    brep_o = nc.declare_dram_parameter("brep_o", [128, D], bf16, isOutput=False)
    wmask = nc.declare_dram_parameter("wmask", [128, NCORES], f32, isOutput=False)
    out_p = nc.declare_dram_parameter("out", [FPC, S, D], f32, isOutput=True)

    with tile.TileContext(nc) as tc:
        with tc.tile_pool(name="wp", bufs=1) as wp, \
             tc.tile_pool(name="pfm", bufs=1) as pfm, \
             tc.tile_pool(name="small", bufs=2) as small, \
             tc.tile_pool(name="pexp", bufs=4) as pexp, \
             tc.tile_pool(name="psA", bufs=4, space="PSUM") as psA, \
             tc.tile_pool(name="psO", bufs=3, space="PSUM") as psO, \
             tc.tile_pool(name="psT", bufs=1, space="PSUM") as psT, \
             tc.tile_pool(name="dram", bufs=1, space="DRAM") as dram:

            # ---- persistent constants ----
            ident = wp.tile([128, 128], bf16, tag="ident")
            make_identity(nc, ident[:])
            ones_row = wp.tile([1, 128], bf16, tag="ones")
            nc.gpsimd.memset(ones_row[:], 1.0)

            bprim_sb = wp.tile([128, NKC], f32, tag="bprim")
            for kc in range(NKC):
                nc.sync.dma_start(out=bprim_sb[:, kc:kc + 1], in_=b_prim[kc * 128:(kc + 1) * 128, None])
            bcq_sb = wp.tile([DH, HEADS], f32, tag="bcq")
            bck_sb = wp.tile([DH, HEADS], f32, tag="bck")
            for h in range(HEADS):
                nc.sync.dma_start(out=bcq_sb[:, h:h + 1], in_=b_cq[DH * h:DH * (h + 1), None])
                nc.sync.dma_start(out=bck_sb[:, h:h + 1], in_=b_ck[DH * h:DH * (h + 1), None])
            brepcv_sb = wp.tile([128, D], bf16, tag="brepcv")
            nc.sync.dma_start(out=brepcv_sb[:], in_=brep_cv[:])
            brepco_sb = wp.tile([128, D], bf16, tag="brepco")
            nc.sync.dma_start(out=brepco_sb[:], in_=brep_co[:])
            brepo_sb = wp.tile([128, D], bf16, tag="brepo")
            nc.sync.dma_start(out=brepo_sb[:], in_=brep_o[:])
            wmask_sb = wp.tile([128, NCORES], f32, tag="wmask")
            nc.sync.dma_start(out=wmask_sb[:], in_=wmask[:])

            wsb = {}
            _weng = [nc.scalar, nc.sync, nc.gpsimd]
            for wi, n in enumerate(["wprim", "wcq", "wck", "wcv", "wq", "wk", "wv"]):
                t = wp.tile([128, NKC, D], bf16, tag=f"w_{n}")
                for kc in range(NKC):
                    _weng[(wi + kc) % 3].dma_start(out=t[:, kc, :], in_=wd[n][kc * 128:(kc + 1) * 128, :])
                wsb[n] = t
            for wi, n in enumerate(["wco", "wo"]):
                t = wp.tile([DH, HEADS, D], bf16, tag=f"w_{n}")
                for h in range(HEADS):
                    _weng[(wi + h) % 3].dma_start(out=t[:, h, :], in_=wd[n][DH * h:DH * (h + 1), :])
                wsb[n] = t

            def load_hs(slot, nm):
                t = pfm.tile([128, NKC, S], bf16, tag="hs", name=nm, bufs=2)
                _he = [nc.scalar, nc.sync]
                for kc in range(NKC):
                    _he[kc % 2].dma_start(out=t[:, kc, :], in_=hsT[slot, kc * 128:(kc + 1) * 128, :])
                return t

            # ---- DRAM scratch ----
            compd = dram.tile([FPC, K, D], f32, tag="compd")
            cc_in = dram.tile([K, D], f32, tag="cc_in")
            cc_out = dram.tile([NCORES, K, D], f32, tag="cc_out")

            # ---- helpers ----
            def proj_fm_head(w, bhead, src, N, out_tag):
                """Per-head FM projection: out [DH, HEADS, N] bf16 = W.T @ src (+bias)."""
                o = pfm.tile([DH, HEADS, N], bf16, tag=out_tag)
                for h in range(HEADS):
                    for n0 in range(0, N, 512):
                        nn = min(512, N - n0)
                        ps = psA.tile([128, 512], f32, tag="psA")
                        for kc in range(NKC):
                            nc.tensor.matmul(
                                out=ps[0:DH, 0:nn],
                                lhsT=w[:, kc, DH * h:DH * (h + 1)],
                                rhs=src[:, kc, n0:n0 + nn],
                                start=(kc == 0), stop=(kc == NKC - 1))
                        if bhead is not None:
                            nc.vector.tensor_scalar(
                                o[:, h, n0:n0 + nn], ps[0:DH, 0:nn],
                                bhead[:, h:h + 1], None, op0=ADD)
                        else:
                            nc.scalar.copy(o[:, h, n0:n0 + nn], ps[0:DH, 0:nn])
                return o

            def proj_tm_v(src, w, brep, frame_tag):
                """TM projection with ones column: list of 8 tiles [128, HEADS, DH+1]."""
                tiles = []
                for tcid in range(NTC):
                    psL = psA.tile([128, 512], f32, tag="psA")
                    psR = psA.tile([128, 512], f32, tag="psA")
                    for kc in range(NKC):
                        nc.tensor.matmul(
                            out=psL[:, 0:512],
                            lhsT=src[:, kc, tcid * 128:(tcid + 1) * 128],
                            rhs=w[:, kc, 0:512],
                            start=(kc == 0), stop=(kc == NKC - 1))
                    for kc in range(NKC):
                        nc.tensor.matmul(
                            out=psR[:, 0:128],
                            lhsT=src[:, kc, tcid * 128:(tcid + 1) * 128],
                            rhs=w[:, kc, 512:640],
                            start=(kc == 0), stop=(kc == NKC - 1))
                    v = pfm.tile([128, HEADS, 97], bf16, tag=f"{frame_tag}{tcid}")
                    nc.vector.memset(v[:, :, DH:96], 0.0)
                    nc.vector.memset(v[:, :, 96:97], 1.0)

                    def _vcopy(hh, d0, d1, srcap, c0):
                        if brep is not None:
                            nc.vector.tensor_tensor(
                                out=v[:, hh, d0:d1], in0=srcap,
                                in1=brep[:, c0:c0 + (d1 - d0)], op=ADD)
                        else:
                            nc.vector.tensor_copy(v[:, hh, d0:d1], srcap)

                    for h in range(6):
                        _vcopy(h, 0, DH, psL[:, DH * h:DH * (h + 1)], DH * h)
                    _vcopy(6, 0, 32, psL[:, 480:512], 480)
                    _vcopy(6, 32, DH, psR[:, 0:48], 512)
                    _vcopy(7, 0, DH, psR[:, 48:128], 560)
                    tiles.append(v)
                return tiles

            def attention(qT, kT, vts, NQ, out_tag):
                """FM attention: O^T [DH, HEADS, NQ] bf16, softmax over S keys."""
                o = pfm.tile([DH, HEADS, NQ], bf16, tag=out_tag)
                pending = []

                def flush_norm():
                    if not pending:
                        return
                    po, h, n0, nn = pending.pop(0)
                    drow_bf = small.tile([1, 512], bf16, tag="drowbf")
                    nc.scalar.copy(out=drow_bf[:, 0:nn], in_=po[96:97, 0:nn])
                    prep = psA.tile([128, 512], f32, tag="psA")
                    nc.tensor.matmul(
                        out=prep[:, 0:nn], lhsT=ones_row[:],
                        rhs=drow_bf[:, 0:nn], start=True, stop=True)
                    recip_sb = pexp.tile([128, 512], f32, tag="recipsb", bufs=2)
                    nc.vector.reciprocal(out=recip_sb[0:DH, 0:nn], in_=prep[0:DH, 0:nn])
                    nc.vector.tensor_tensor(
                        out=o[:, h, n0:n0 + nn], in0=po[0:DH, 0:nn],
                        in1=recip_sb[0:DH, 0:nn], op=MUL)

                for h in range(HEADS):
                    for n0 in range(0, NQ, 512):
                        nn = min(512, NQ - n0)
                        po = psO.tile([128, 512], f32, tag="psO")
                        for tcid in range(NTC):
                            ps = psA.tile([128, 512], f32, tag="psA")
                            nc.tensor.matmul(
                                out=ps[:, 0:nn],
                                lhsT=kT[:, h, tcid * 128:(tcid + 1) * 128],
                                rhs=qT[:, h, n0:n0 + nn],
                                start=True, stop=True)
                            ex = pexp.tile([128, 512], bf16, tag="exp", bufs=3)
                            nc.scalar.activation(
                                out=ex[:, 0:nn], in_=ps[:, 0:nn],
                                func=mybir.ActivationFunctionType.Exp)
                            nc.tensor.matmul(
                                out=po[0:97, 0:nn],
                                lhsT=vts[tcid][:, h, 0:97],
                                rhs=ex[:, 0:nn],
                                start=(tcid == 0), stop=(tcid == NTC - 1))
                        pending.append((po, h, n0, nn))
                        if len(pending) > 1:
                            flush_norm()
                while pending:
                    flush_norm()
                return o

            # ================= COMP BRANCH (both frames first) =================
            ssum = [pfm.tile([128, D], f32, tag=f"ssum{sc}", name=f"ssum{sc}") for sc in range(NSC)]
            hs_tiles = []
            for f in range(FPC):
                hs_prev = load_hs(f, f"hsc{f}")
                hs_tiles.append(hs_prev)
                g1 = pfm.tile([128, NKC, K], bf16, tag="g1")
                g2 = pfm.tile([128, NKC, K], bf16, tag="g2")
                _ge = nc.gpsimd if f == 0 else nc.sync
                for kc in range(NKC):
                    _ge.dma_start(out=g1[:, kc, :], in_=g1t[f, kc * 128:(kc + 1) * 128, :])
                    _ge.dma_start(out=g2[:, kc, :], in_=g2t[f, kc * 128:(kc + 1) * 128, :])
                gd = pfm.tile([128, NKC, K], bf16, tag="zt")
                nc.vector.tensor_tensor(out=gd[:], in0=g1[:], in1=g2[:], op=SUB)

                # primary_hs (FM, chunk-aligned): prim = gd @ Wprim + b + g1
                prim = pfm.tile([128, NKC, K], bf16, tag="prim")
                for mc in range(NKC):
                    ps = psA.tile([128, 512], f32, tag="psA")
                    for kc in range(NKC):
                        nc.tensor.matmul(
                            out=ps[:], lhsT=wsb["wprim"][:, kc, mc * 128:(mc + 1) * 128],
                            rhs=gd[:, kc, :], start=(kc == 0), stop=(kc == NKC - 1))
                    nc.vector.tensor_scalar(
                        prim[:, mc, :], ps[:], bprim_sb[:, mc:mc + 1], None, op0=ADD)
                    nc.vector.tensor_tensor(
                        out=prim[:, mc, :], in0=prim[:, mc, :], in1=g1[:, mc, :], op=ADD)

                qcT = proj_fm_head(wsb["wcq"], bcq_sb, prim, K, "qT")
                kcT = proj_fm_head(wsb["wck"], bck_sb, hs_prev, S, "kT")
                vC = proj_tm_v(hs_prev, wsb["wcv"], brepcv_sb, "v")
                ocT = attention(qcT, kcT, vC, K, "oT")

                # comp_out projection (TM) + bias; spill to DRAM + local sum
                for sc in range(NSC):
                    psL = psA.tile([128, 512], f32, tag="psA")
                    psR = psA.tile([128, 512], f32, tag="psA")
                    for h in range(HEADS):
                        nc.tensor.matmul(
                            out=psL[:, 0:512],
                            lhsT=ocT[:, h, sc * 128:(sc + 1) * 128],
                            rhs=wsb["wco"][:, h, 0:512],
                            start=(h == 0), stop=(h == HEADS - 1))
                    for h in range(HEADS):
                        nc.tensor.matmul(
                            out=psR[:, 0:128],
                            lhsT=ocT[:, h, sc * 128:(sc + 1) * 128],
                            rhs=wsb["wco"][:, h, 512:640],
                            start=(h == 0), stop=(h == HEADS - 1))
                    csb = small.tile([128, D], f32, tag="wk640")
                    nc.vector.tensor_tensor(out=csb[:, 0:512], in0=psL[:, 0:512],
                                            in1=brepco_sb[:, 0:512], op=ADD)
                    nc.vector.tensor_tensor(out=csb[:, 512:640], in0=psR[:, 0:128],
                                            in1=brepco_sb[:, 512:640], op=ADD)
                    nc.sync.dma_start(out=compd[f, sc * 128:(sc + 1) * 128, :], in_=csb[:])
                    if f == 0:
                        nc.vector.tensor_copy(ssum[sc][:], csb[:])
                    else:
                        nc.vector.tensor_tensor(out=ssum[sc][:], in0=ssum[sc][:], in1=csb[:], op=ADD)

            for sc in range(NSC):
                nc.sync.dma_start(out=cc_in[sc * 128:(sc + 1) * 128, :], in_=ssum[sc][:])
            nc.gpsimd.collective_compute(
                "AllGather", mybir.AluOpType.bypass,
                replica_groups=[list(range(NCORES))],
                ins=[cc_in.opt()], outs=[cc_out.opt()])

            # masked prefix, entirely on the gpsimd queue (inherently ordered
            # after the collective; keeps the PE/DVE/sync queues unblocked)
            pfx = [pfm.tile([128, D], f32, tag=f"ssum{sc}", name=f"pfx{sc}")
                   for sc in range(NSC)]
            for sc in range(NSC):
                nc.gpsimd.memset(pfx[sc][:], 0.0)
                for cc in range(NCORES):
                    tmp = small.tile([128, D], f32, tag="wk640b")
                    nc.gpsimd.dma_start(out=tmp[:], in_=cc_out[cc, sc * 128:(sc + 1) * 128, :])
                    nc.gpsimd.tensor_tensor(
                        out=tmp[:], in0=tmp[:],
                        in1=wmask_sb[:, cc:cc + 1].to_broadcast([128, D]), op=MUL)
                    nc.gpsimd.tensor_tensor(out=pfx[sc][:], in0=pfx[sc][:], in1=tmp[:], op=ADD)

            # ================= MAIN BRANCH =================
            for f in range(FPC):
                if f == 0:
                    hs_prev = hs_tiles[0]
                    hs_own = hs_tiles[1]
                else:
                    hs_prev = hs_tiles[1]
                    hs_own = load_hs(2, "hsm2")
                q2T = proj_fm_head(wsb["wq"], None, hs_own, S, "qT")
                k2T = proj_fm_head(wsb["wk"], None, hs_prev, S, "kT")
                v2 = proj_tm_v(hs_prev, wsb["wv"], None, "v")
                o2T = attention(q2T, k2T, v2, S, "oT")

                # out projection -> Z^T bf16
                zt = pfm.tile([128, NKC, S], bf16, tag="zt")
                for mc in range(NKC):
                    for n0 in range(0, S, 512):
                        ps = psA.tile([128, 512], f32, tag="psA")
                        for h in range(HEADS):
                            nc.tensor.matmul(
                                out=ps[:],
                                lhsT=wsb["wo"][:, h, mc * 128:(mc + 1) * 128],
                                rhs=o2T[:, h, n0:n0 + 512],
                                start=(h == 0), stop=(h == HEADS - 1))
                        nc.scalar.copy(out=zt[:, mc, n0:n0 + 512], in_=ps[:])

                # transpose to TM, add bias, store
                for tcid in range(NTC):
                    pt = psT.tile([128, D], bf16, tag="psT")
                    for mc in range(NKC):
                        nc.tensor.transpose(
                            out=pt[:, mc * 128:(mc + 1) * 128],
                            in_=zt[:, mc, tcid * 128:(tcid + 1) * 128],
                            identity=ident[:])
                    zfin = small.tile([128, D], f32, tag="wk640")
                    nc.vector.tensor_tensor(out=zfin[:], in0=pt[:], in1=brepo_sb[:], op=ADD)
                    nc.sync.dma_start(out=out_p[f, tcid * 128:(tcid + 1) * 128, :], in_=zfin[:])



            # cumsum + scatter-add RMW, entirely on the gpsimd queue;
            # phased per frame so the 4 indirect gathers pipeline their transfers
            for f in range(FPC):
                opv = out_p[0:1] if f == 0 else out_p[:]
                idxs = []
                for sc in range(NSC):
                    ctile = small.tile([128, D], f32, tag="wk640b")
                    nc.gpsimd.dma_start(out=ctile[:], in_=compd[f, sc * 128:(sc + 1) * 128, :])
                    nc.gpsimd.tensor_tensor(out=pfx[sc][:], in0=pfx[sc][:], in1=ctile[:], op=ADD)
                    idx = small.tile([128, 1], i32, tag="idx8", bufs=8)
                    nc.gpsimd.dma_start(out=idx[:], in_=topk[f, sc * 128:(sc + 1) * 128, None])
                    idxs.append(idx)
                for p0 in range(0, NSC, 2):
                    pair = range(p0, min(p0 + 2, NSC))
                    pg = {}
                    for sc in pair:
                        grow = small.tile([128, D], f32, tag="grow4", bufs=2)
                        nc.gpsimd.indirect_dma_start(
                            out=grow[:], out_offset=None, in_=opv,
                            in_offset=bass.IndirectOffsetOnAxis(ap=idxs[sc][:, :1], axis=1),
                            element_offset=f * S * D)
                        pg[sc] = grow
                    for sc in pair:
                        nc.gpsimd.tensor_tensor(out=pg[sc][:], in0=pg[sc][:], in1=pfx[sc][:], op=ADD)
                    for sc in pair:
                        nc.gpsimd.indirect_dma_start(
                            out=opv, out_offset=bass.IndirectOffsetOnAxis(ap=idxs[sc][:, :1], axis=1),
                            in_=pg[sc][:], in_offset=None,
                            element_offset=f * S * D)

    nc.compile()
    return nc


def _host_prep(hidden_states, topk_idx, weights):
    """Build the 8 per-core input maps (pure data movement + dtype casts)."""
    bf = ml_dtypes.bfloat16
    hs = np.asarray(hidden_states, np.float32)
    (wq, wk, wv, wo, wcq, wck, wcv, wco, wprim,
     b_prim, b_cq, b_ck, b_cv, b_co, b_o) = weights

    wq_s = (wq * SCALE).astype(bf)
    wcq_s = (wcq * SCALE).astype(bf)
    b_cq_s = (b_cq * SCALE).astype(np.float32)
    wcast = {
        "wq": wq_s, "wk": wk.astype(bf), "wv": wv.astype(bf), "wo": wo.astype(bf),
        "wcq": wcq_s, "wck": wck.astype(bf), "wcv": wcv.astype(bf),
        "wco": wco.astype(bf), "wprim": wprim.astype(bf),
    }
    shared = dict(wcast)
    shared["b_prim"] = b_prim.astype(np.float32)
    shared["b_cq"] = b_cq_s
    shared["b_ck"] = b_ck.astype(np.float32)
    shared["brep_cv"] = np.broadcast_to(b_cv.astype(bf), (128, D)).copy()
    shared["brep_co"] = np.broadcast_to(b_co.astype(bf), (128, D)).copy()
    shared["brep_o"] = np.broadcast_to(b_o.astype(bf), (128, D)).copy()

    in_maps = []
    for c in range(NCORES):
        f0, f1 = 2 * c, 2 * c + 1
        p0 = max(f0 - 1, 0)
        m = dict(shared)
        m["hsT"] = np.ascontiguousarray(
            hs[[p0, f0, f1]].transpose(0, 2, 1)).astype(bf)
        g1 = np.stack([hs[f0][topk_idx[f0]], hs[f1][topk_idx[f1]]])
        g2 = np.stack([hs[p0][topk_idx[f0]], hs[f0][topk_idx[f1]]])
        m["g1t"] = np.ascontiguousarray(g1.transpose(0, 2, 1)).astype(bf)
        m["g2t"] = np.ascontiguousarray(g2.transpose(0, 2, 1)).astype(bf)
        m["topk"] = np.ascontiguousarray(topk_idx[[f0, f1]]).astype(np.int32)
        wm = np.zeros((128, NCORES), np.float32)
        wm[:, :c] = 1.0
        m["wmask"] = wm
        in_maps.append(m)
    return in_maps


def kernel(hidden_states, primary_lin_w, primary_lin_b,
           comp_q_w, comp_q_b, comp_k_w, comp_k_b, comp_v_w, comp_v_b,
           comp_out_w, comp_out_b,
           to_q_w, to_k_w, to_v_w, to_out_w, to_out_b,
           video_length):
    import jax
    import jax.numpy as jnp
    from concourse.bass_utils import run_bass_kernel_spmd

    f = int(video_length)
    assert f == F and hidden_states.shape == (F, S, D)

    # Top-k selection: replicate the reference's exact eager-jax computation
    # (selection order is rounding-critical; must match bitwise).
    hs_j = jnp.asarray(np.asarray(hidden_states, np.float32))
    pre = jnp.concatenate([hs_j[:1], hs_j[:-1]], axis=0)
    diff_score = jnp.abs(hs_j - pre).mean(axis=-1)
    _, topk_idx = jax.lax.top_k(diff_score, max(64, S // 2))
    topk_idx = np.asarray(topk_idx)

    weights = tuple(np.asarray(w, np.float32) for w in (
        to_q_w, to_k_w, to_v_w, to_out_w, comp_q_w, comp_k_w, comp_v_w,
        comp_out_w, primary_lin_w, primary_lin_b, comp_q_b, comp_k_b,
        comp_v_b, comp_out_b, to_out_b))
    in_maps = _host_prep(hidden_states, topk_idx, weights)

    if "nc" not in _cache:
        _cache["nc"] = _build_nc()
    res = run_bass_kernel_spmd(_cache["nc"], in_maps, list(range(NCORES)))
    out = np.concatenate([res.results[c]["out"] for c in range(NCORES)], axis=0)
    return np.ascontiguousarray(out.astype(np.float32))


# revision 32
# speedup vs baseline: 1.1827x; 1.0956x over previous
"""Trainium2 Bass kernel for nn_BasicTransformerBlock_50208167690869.

Sparse-attention transformer block, sharded data-parallel over the 16-frame
axis across 8 NeuronCores (2 frames/core, 1-frame halo for the per-frame
shift). The cross-frame cumsum of the compression branch is realized with an
on-device AllGather of each core's local comp sum plus a masked prefix
reduction. Top-k selection is computed on host with the exact same jax ops
as the reference (bitwise-identical selection; the ranking is
rounding-critical), everything else runs on device.

Self-contained: hardcodes shapes from the problem spec.
"""
import sys

for _p in ("/opt/trn_rl_repo",):
    if _p not in sys.path:
        sys.path.append(_p)

import numpy as np
import ml_dtypes

HEADS = 8
DH = 80
D = 640
F = 16
S = 1024
K = 512
NCORES = 8
FPC = F // NCORES  # frames per core
SCALE = DH ** -0.5
NKC = D // 128     # 5 contraction chunks
NTC = S // 128     # 8 token chunks
NSC = K // 128     # 4 slot chunks

_cache = {}


def _apply_drain_patch():
    """This walrus build rejects >1 sync-wait on CTRL instructions; split the
    TileContext tail-drain waits across single-wait nops."""
    import concourse.tile as tile
    from concourse.vector_clock import ScopedClock, VectorClock

    if getattr(tile.TileContext, "_drain_patched", False):
        return

    def _patched(self, tick_clock, wait_clock):
        nc = self.nc
        gc = tick_clock.global_clock
        n = len(gc)
        for p in range(n):
            t = gc[p]
            if t == 0:
                continue
            vc = VectorClock([t if i == p else 0 for i in range(n)])
            nop_inst = nc.sync.nop()
            wait_clock.add_sem_waits(nop_inst.ins, ScopedClock({None: vc}))
        nc.sync.drain()
        nc.all_engine_barrier()
        assert self.sems is not None
        popped = nc._tile_sem_poison_stack.pop()
        assert popped is self._sem_poison
        nc.clear_and_free_semaphores(list(self.sems.allocated().values()))
        nc.all_engine_barrier()

    tile.TileContext._drain_and_barrier = _patched
    tile.TileContext._drain_patched = True


def _build_nc():
    import concourse.bass as bass
    import concourse.bacc as bacc
    import concourse.mybir as mybir
    import concourse.tile as tile
    from concourse.masks import make_identity

    _apply_drain_patch()

    bf16 = mybir.dt.bfloat16
    f32 = mybir.dt.float32
    i32 = mybir.dt.int32
    ADD = mybir.AluOpType.add
    SUB = mybir.AluOpType.subtract
    MUL = mybir.AluOpType.mult

    nc = bacc.Bacc("TRN2", target_bir_lowering=False, num_devices=NCORES)

    # ---- params ----
    hsT = nc.declare_dram_parameter("hsT", [3, D, S], bf16, isOutput=False)
    g1t = nc.declare_dram_parameter("g1t", [FPC, D, K], bf16, isOutput=False)
    g2t = nc.declare_dram_parameter("g2t", [FPC, D, K], bf16, isOutput=False)
    topk = nc.declare_dram_parameter("topk", [FPC, K], i32, isOutput=False)
    wnames = ["wq", "wk", "wv", "wo", "wcq", "wck", "wcv", "wco", "wprim"]
    wd = {n: nc.declare_dram_parameter(n, [D, D], bf16, isOutput=False) for n in wnames}
    b_prim = nc.declare_dram_parameter("b_prim", [D], f32, isOutput=False)
    b_cq = nc.declare_dram_parameter("b_cq", [D], f32, isOutput=False)
    b_ck = nc.declare_dram_parameter("b_ck", [D], f32, isOutput=False)
    brep_cv = nc.declare_dram_parameter("brep_cv", [128, D], bf16, isOutput=False)
    brep_co = nc.declare_dram_parameter("brep_co", [128, D], bf16, isOutput=False)
    brep_o = nc.declare_dram_parameter("brep_o", [128, D], bf16, isOutput=False)
    wmask = nc.declare_dram_parameter("wmask", [128, NCORES], f32, isOutput=False)
    out_p = nc.declare_dram_parameter("out", [FPC, S, D], f32, isOutput=True)

    with tile.TileContext(nc) as tc:
        with tc.tile_pool(name="wp", bufs=1) as wp, \
             tc.tile_pool(name="pfm", bufs=1) as pfm, \
             tc.tile_pool(name="small", bufs=2) as small, \
             tc.tile_pool(name="pexp", bufs=4) as pexp, \
             tc.tile_pool(name="psA", bufs=4, space="PSUM") as psA, \
             tc.tile_pool(name="psO", bufs=3, space="PSUM") as psO, \
             tc.tile_pool(name="psT", bufs=1, space="PSUM") as psT, \
             tc.tile_pool(name="dram", bufs=1, space="DRAM") as dram:

            # ---- persistent constants ----
            ident = wp.tile([128, 128], bf16, tag="ident")
            make_identity(nc, ident[:])
            ones_row = wp.tile([1, 128], bf16, tag="ones")
            nc.gpsimd.memset(ones_row[:], 1.0)

            bprim_sb = wp.tile([128, NKC], f32, tag="bprim")
            for kc in range(NKC):
                nc.sync.dma_start(out=bprim_sb[:, kc:kc + 1], in_=b_prim[kc * 128:(kc + 1) * 128, None])
            bcq_sb = wp.tile([DH, HEADS], f32, tag="bcq")
            bck_sb = wp.tile([DH, HEADS], f32, tag="bck")
            for h in range(HEADS):
                nc.sync.dma_start(out=bcq_sb[:, h:h + 1], in_=b_cq[DH * h:DH * (h + 1), None])
                nc.sync.dma_start(out=bck_sb[:, h:h + 1], in_=b_ck[DH * h:DH * (h + 1), None])
            brepcv_sb = wp.tile([128, D], bf16, tag="brepcv")
            nc.sync.dma_start(out=brepcv_sb[:], in_=brep_cv[:])
            brepco_sb = wp.tile([128, D], bf16, tag="brepco")
            nc.sync.dma_start(out=brepco_sb[:], in_=brep_co[:])
            brepo_sb = wp.tile([128, D], bf16, tag="brepo")
            nc.sync.dma_start(out=brepo_sb[:], in_=brep_o[:])
            wmask_sb = wp.tile([128, NCORES], f32, tag="wmask")
            nc.sync.dma_start(out=wmask_sb[:], in_=wmask[:])

            wsb = {}
            _weng = [nc.scalar, nc.sync, nc.gpsimd]
            for wi, n in enumerate(["wprim", "wcq", "wck", "wcv", "wq", "wk", "wv"]):
                t = wp.tile([128, NKC, D], bf16, tag=f"w_{n}")
                for kc in range(NKC):
                    _weng[(wi + kc) % 3].dma_start(out=t[:, kc, :], in_=wd[n][kc * 128:(kc + 1) * 128, :])
                wsb[n] = t
            for wi, n in enumerate(["wco", "wo"]):
                t = wp.tile([DH, HEADS, D], bf16, tag=f"w_{n}")
                for h in range(HEADS):
                    _weng[(wi + h) % 3].dma_start(out=t[:, h, :], in_=wd[n][DH * h:DH * (h + 1), :])
                wsb[n] = t

            def load_hs(slot, nm):
                t = pfm.tile([128, NKC, S], bf16, tag="hs", name=nm, bufs=2)
                _he = [nc.scalar, nc.sync]
                for kc in range(NKC):
                    _he[kc % 2].dma_start(out=t[:, kc, :], in_=hsT[slot, kc * 128:(kc + 1) * 128, :])
                return t

            # ---- DRAM scratch ----
            compd = dram.tile([FPC, K, D], f32, tag="compd")
            cc_in = dram.tile([K, D], f32, tag="cc_in")
            cc_out = dram.tile([NCORES, K, D], f32, tag="cc_out")

            # ---- helpers ----
            def proj_fm_head(w, bhead, src, N, out_tag):
                """Per-head FM projection: out [DH, HEADS, N] bf16 = W.T @ src (+bias)."""
                o = pfm.tile([DH, HEADS, N], bf16, tag=out_tag)
                for h in range(HEADS):
                    for n0 in range(0, N, 512):
                        nn = min(512, N - n0)
                        ps = psA.tile([128, 512], f32, tag="psA")
                        for kc in range(NKC):
                            nc.tensor.matmul(
                                out=ps[0:DH, 0:nn],
                                lhsT=w[:, kc, DH * h:DH * (h + 1)],
                                rhs=src[:, kc, n0:n0 + nn],
                                start=(kc == 0), stop=(kc == NKC - 1))
                        if bhead is not None:
                            nc.vector.tensor_scalar(
                                o[:, h, n0:n0 + nn], ps[0:DH, 0:nn],
                                bhead[:, h:h + 1], None, op0=ADD)
                        else:
                            nc.scalar.copy(o[:, h, n0:n0 + nn], ps[0:DH, 0:nn])
                return o

            def proj_tm_v(src, w, brep, frame_tag):
                """TM projection with ones column: list of 8 tiles [128, HEADS, DH+1]."""
                tiles = []
                for tcid in range(NTC):
                    psL = psA.tile([128, 512], f32, tag="psA")
                    psR = psA.tile([128, 512], f32, tag="psA")
                    for kc in range(NKC):
                        nc.tensor.matmul(
                            out=psL[:, 0:512],
                            lhsT=src[:, kc, tcid * 128:(tcid + 1) * 128],
                            rhs=w[:, kc, 0:512],
                            start=(kc == 0), stop=(kc == NKC - 1))
                    for kc in range(NKC):
                        nc.tensor.matmul(
                            out=psR[:, 0:128],
                            lhsT=src[:, kc, tcid * 128:(tcid + 1) * 128],
                            rhs=w[:, kc, 512:640],
                            start=(kc == 0), stop=(kc == NKC - 1))
                    v = pfm.tile([128, HEADS, 97], bf16, tag=f"{frame_tag}{tcid}")
                    nc.vector.memset(v[:, :, DH:96], 0.0)
                    nc.vector.memset(v[:, :, 96:97], 1.0)

                    def _vcopy(hh, d0, d1, srcap, c0):
                        if brep is not None:
                            nc.vector.tensor_tensor(
                                out=v[:, hh, d0:d1], in0=srcap,
                                in1=brep[:, c0:c0 + (d1 - d0)], op=ADD)
                        else:
                            nc.vector.tensor_copy(v[:, hh, d0:d1], srcap)

                    for h in range(6):
                        _vcopy(h, 0, DH, psL[:, DH * h:DH * (h + 1)], DH * h)
                    _vcopy(6, 0, 32, psL[:, 480:512], 480)
                    _vcopy(6, 32, DH, psR[:, 0:48], 512)
                    _vcopy(7, 0, DH, psR[:, 48:128], 560)
                    tiles.append(v)
                return tiles

            def proj_head_into(w, bhead, src, N, otile, h):
                for n0 in range(0, N, 512):
                    nn = min(512, N - n0)
                    ps = psA.tile([128, 512], f32, tag="psA")
                    for kc in range(NKC):
                        nc.tensor.matmul(
                            out=ps[0:DH, 0:nn],
                            lhsT=w[:, kc, DH * h:DH * (h + 1)],
                            rhs=src[:, kc, n0:n0 + nn],
                            start=(kc == 0), stop=(kc == NKC - 1))
                    if bhead is not None:
                        nc.vector.tensor_scalar(
                            otile[:, h, n0:n0 + nn], ps[0:DH, 0:nn],
                            bhead[:, h:h + 1], None, op0=ADD)
                    else:
                        nc.scalar.copy(otile[:, h, n0:n0 + nn], ps[0:DH, 0:nn])

            def fused_branch(qw, qb, ksrcw, kb, qsrc, ksrc, vts, NQ,
                             qtag, ktag, out_tag):
                """Per-head pipelined projection + attention: head h+1's q/k
                projections are emitted before head h's attention units so the
                PE->DVE/ACT copy handoff is hidden behind matmul work."""
                qT = pfm.tile([DH, HEADS, NQ], bf16, tag=qtag, name=qtag + "t")
                kT = pfm.tile([DH, HEADS, S], bf16, tag=ktag, name=ktag + "t")
                o = pfm.tile([DH, HEADS, NQ], bf16, tag=out_tag, name=out_tag + "t")
                pending = []

                def proj_head(h):
                    proj_head_into(qw, qb, qsrc, NQ, qT, h)
                    proj_head_into(ksrcw, kb, ksrc, S, kT, h)

                def flush_norm():
                    if not pending:
                        return
                    po, h, n0, nn = pending.pop(0)
                    drow_bf = small.tile([1, 512], bf16, tag="drowbf")
                    nc.scalar.copy(out=drow_bf[:, 0:nn], in_=po[96:97, 0:nn])
                    prep = psA.tile([128, 512], f32, tag="psA")
                    nc.tensor.matmul(
                        out=prep[:, 0:nn], lhsT=ones_row[:],
                        rhs=drow_bf[:, 0:nn], start=True, stop=True)
                    recip_sb = pexp.tile([128, 512], f32, tag="recipsb", bufs=2)
                    nc.vector.reciprocal(out=recip_sb[0:DH, 0:nn], in_=prep[0:DH, 0:nn])
                    nc.vector.tensor_tensor(
                        out=o[:, h, n0:n0 + nn], in0=po[0:DH, 0:nn],
                        in1=recip_sb[0:DH, 0:nn], op=MUL)

                proj_head(0)
                for h in range(HEADS):
                    if h + 1 < HEADS:
                        proj_head(h + 1)
                    for n0 in range(0, NQ, 512):
                        nn = min(512, NQ - n0)
                        po = psO.tile([128, 512], f32, tag="psO")
                        for tcid in range(NTC):
                            ps = psA.tile([128, 512], f32, tag="psA")
                            nc.tensor.matmul(
                                out=ps[:, 0:nn],
                                lhsT=kT[:, h, tcid * 128:(tcid + 1) * 128],
                                rhs=qT[:, h, n0:n0 + nn],
                                start=True, stop=True)
                            ex = pexp.tile([128, 512], bf16, tag="exp", bufs=3)
                            nc.scalar.activation(
                                out=ex[:, 0:nn], in_=ps[:, 0:nn],
                                func=mybir.ActivationFunctionType.Exp)
                            nc.tensor.matmul(
                                out=po[0:97, 0:nn],
                                lhsT=vts[tcid][:, h, 0:97],
                                rhs=ex[:, 0:nn],
                                start=(tcid == 0), stop=(tcid == NTC - 1))
                        pending.append((po, h, n0, nn))
                        if len(pending) > 1:
                            flush_norm()
                while pending:
                    flush_norm()
                return o

            def attention(qT, kT, vts, NQ, out_tag):
                """FM attention: O^T [DH, HEADS, NQ] bf16, softmax over S keys."""
                o = pfm.tile([DH, HEADS, NQ], bf16, tag=out_tag)
                pending = []

                def flush_norm():
                    if not pending:
                        return
                    po, h, n0, nn = pending.pop(0)
                    drow_bf = small.tile([1, 512], bf16, tag="drowbf")
                    nc.scalar.copy(out=drow_bf[:, 0:nn], in_=po[96:97, 0:nn])
                    prep = psA.tile([128, 512], f32, tag="psA")
                    nc.tensor.matmul(
                        out=prep[:, 0:nn], lhsT=ones_row[:],
                        rhs=drow_bf[:, 0:nn], start=True, stop=True)
                    recip_sb = pexp.tile([128, 512], f32, tag="recipsb", bufs=2)
                    nc.vector.reciprocal(out=recip_sb[0:DH, 0:nn], in_=prep[0:DH, 0:nn])
                    nc.vector.tensor_tensor(
                        out=o[:, h, n0:n0 + nn], in0=po[0:DH, 0:nn],
                        in1=recip_sb[0:DH, 0:nn], op=MUL)

                for h in range(HEADS):
                    for n0 in range(0, NQ, 512):
                        nn = min(512, NQ - n0)
                        po = psO.tile([128, 512], f32, tag="psO")
                        for tcid in range(NTC):
                            ps = psA.tile([128, 512], f32, tag="psA")
                            nc.tensor.matmul(
                                out=ps[:, 0:nn],
                                lhsT=kT[:, h, tcid * 128:(tcid + 1) * 128],
                                rhs=qT[:, h, n0:n0 + nn],
                                start=True, stop=True)
                            ex = pexp.tile([128, 512], bf16, tag="exp", bufs=3)
                            nc.scalar.activation(
                                out=ex[:, 0:nn], in_=ps[:, 0:nn],
                                func=mybir.ActivationFunctionType.Exp)
                            nc.tensor.matmul(
                                out=po[0:97, 0:nn],
                                lhsT=vts[tcid][:, h, 0:97],
                                rhs=ex[:, 0:nn],
                                start=(tcid == 0), stop=(tcid == NTC - 1))
                        pending.append((po, h, n0, nn))
                        if len(pending) > 1:
                            flush_norm()
                while pending:
                    flush_norm()
                return o

            # ================= COMP BRANCH (both frames first) =================
            ssum = [pfm.tile([128, D], f32, tag=f"ssum{sc}", name=f"ssum{sc}") for sc in range(NSC)]
            hs_tiles = []
            for f in range(FPC):
                hs_prev = load_hs(f, f"hsc{f}")
                hs_tiles.append(hs_prev)
                g1 = pfm.tile([128, NKC, K], bf16, tag="g1")
                g2 = pfm.tile([128, NKC, K], bf16, tag="g2")
                _ge = nc.gpsimd if f == 0 else nc.sync
                for kc in range(NKC):
                    _ge.dma_start(out=g1[:, kc, :], in_=g1t[f, kc * 128:(kc + 1) * 128, :])
                    _ge.dma_start(out=g2[:, kc, :], in_=g2t[f, kc * 128:(kc + 1) * 128, :])
                gd = pfm.tile([128, NKC, K], bf16, tag="zt")
                nc.vector.tensor_tensor(out=gd[:], in0=g1[:], in1=g2[:], op=SUB)

                # primary_hs (FM, chunk-aligned): prim = gd @ Wprim + b + g1
                prim = pfm.tile([128, NKC, K], bf16, tag="prim")
                for mc in range(NKC):
                    ps = psA.tile([128, 512], f32, tag="psA")
                    for kc in range(NKC):
                        nc.tensor.matmul(
                            out=ps[:], lhsT=wsb["wprim"][:, kc, mc * 128:(mc + 1) * 128],
                            rhs=gd[:, kc, :], start=(kc == 0), stop=(kc == NKC - 1))
                    nc.vector.tensor_scalar(
                        prim[:, mc, :], ps[:], bprim_sb[:, mc:mc + 1], None, op0=ADD)
                    nc.vector.tensor_tensor(
                        out=prim[:, mc, :], in0=prim[:, mc, :], in1=g1[:, mc, :], op=ADD)

                vC = proj_tm_v(hs_prev, wsb["wcv"], brepcv_sb, "v")
                ocT = fused_branch(wsb["wcq"], bcq_sb, wsb["wck"], bck_sb,
                                   prim, hs_prev, vC, K, "qT", "kT", "oT")

                # comp_out projection (TM) + bias; spill to DRAM + local sum
                for sc in range(NSC):
                    psL = psA.tile([128, 512], f32, tag="psA")
                    psR = psA.tile([128, 512], f32, tag="psA")
                    for h in range(HEADS):
                        nc.tensor.matmul(
                            out=psL[:, 0:512],
                            lhsT=ocT[:, h, sc * 128:(sc + 1) * 128],
                            rhs=wsb["wco"][:, h, 0:512],
                            start=(h == 0), stop=(h == HEADS - 1))
                    for h in range(HEADS):
                        nc.tensor.matmul(
                            out=psR[:, 0:128],
                            lhsT=ocT[:, h, sc * 128:(sc + 1) * 128],
                            rhs=wsb["wco"][:, h, 512:640],
                            start=(h == 0), stop=(h == HEADS - 1))
                    csb = small.tile([128, D], f32, tag="wk640")
                    nc.vector.tensor_tensor(out=csb[:, 0:512], in0=psL[:, 0:512],
                                            in1=brepco_sb[:, 0:512], op=ADD)
                    nc.vector.tensor_tensor(out=csb[:, 512:640], in0=psR[:, 0:128],
                                            in1=brepco_sb[:, 512:640], op=ADD)
                    nc.sync.dma_start(out=compd[f, sc * 128:(sc + 1) * 128, :], in_=csb[:])
                    if f == 0:
                        nc.vector.tensor_copy(ssum[sc][:], csb[:])
                    else:
                        nc.vector.tensor_tensor(out=ssum[sc][:], in0=ssum[sc][:], in1=csb[:], op=ADD)

            for sc in range(NSC):
                nc.sync.dma_start(out=cc_in[sc * 128:(sc + 1) * 128, :], in_=ssum[sc][:])
            nc.gpsimd.collective_compute(
                "AllGather", mybir.AluOpType.bypass,
                replica_groups=[list(range(NCORES))],
                ins=[cc_in.opt()], outs=[cc_out.opt()])

            # masked prefix, entirely on the gpsimd queue (inherently ordered
            # after the collective; keeps the PE/DVE/sync queues unblocked)
            pfx = [pfm.tile([128, D], f32, tag=f"ssum{sc}", name=f"pfx{sc}")
                   for sc in range(NSC)]
            for sc in range(NSC):
                nc.gpsimd.memset(pfx[sc][:], 0.0)
                for cc in range(NCORES):
                    tmp = small.tile([128, D], f32, tag="wk640b")
                    nc.gpsimd.dma_start(out=tmp[:], in_=cc_out[cc, sc * 128:(sc + 1) * 128, :])
                    nc.gpsimd.tensor_tensor(
                        out=tmp[:], in0=tmp[:],
                        in1=wmask_sb[:, cc:cc + 1].to_broadcast([128, D]), op=MUL)
                    nc.gpsimd.tensor_tensor(out=pfx[sc][:], in0=pfx[sc][:], in1=tmp[:], op=ADD)

            # ================= MAIN BRANCH =================
            for f in range(FPC):
                if f == 0:
                    hs_prev = hs_tiles[0]
                    hs_own = hs_tiles[1]
                else:
                    hs_prev = hs_tiles[1]
                    hs_own = load_hs(2, "hsm2")
                v2 = proj_tm_v(hs_prev, wsb["wv"], None, "v")
                o2T = fused_branch(wsb["wq"], None, wsb["wk"], None,
                                   hs_own, hs_prev, v2, S, "qT", "kT", "oT")

                # out projection -> Z^T bf16
                zt = pfm.tile([128, NKC, S], bf16, tag="zt")
                for mc in range(NKC):
                    for n0 in range(0, S, 512):
                        ps = psA.tile([128, 512], f32, tag="psA")
                        for h in range(HEADS):
                            nc.tensor.matmul(
                                out=ps[:],
                                lhsT=wsb["wo"][:, h, mc * 128:(mc + 1) * 128],
                                rhs=o2T[:, h, n0:n0 + 512],
                                start=(h == 0), stop=(h == HEADS - 1))
                        nc.scalar.copy(out=zt[:, mc, n0:n0 + 512], in_=ps[:])

                # transpose to TM, add bias, store
                for tcid in range(NTC):
                    pt = psT.tile([128, D], bf16, tag="psT")
                    for mc in range(NKC):
                        nc.tensor.transpose(
                            out=pt[:, mc * 128:(mc + 1) * 128],
                            in_=zt[:, mc, tcid * 128:(tcid + 1) * 128],
                            identity=ident[:])
                    zfin = small.tile([128, D], f32, tag="wk640")
                    nc.vector.tensor_tensor(out=zfin[:], in0=pt[:], in1=brepo_sb[:], op=ADD)
                    nc.sync.dma_start(out=out_p[f, tcid * 128:(tcid + 1) * 128, :], in_=zfin[:])



            # cumsum + scatter-add RMW, entirely on the gpsimd queue;
            # phased per frame so the 4 indirect gathers pipeline their transfers
            for f in range(FPC):
                opv = out_p[0:1] if f == 0 else out_p[:]
                idxs = []
                for sc in range(NSC):
                    ctile = small.tile([128, D], f32, tag="wk640b")
                    nc.gpsimd.dma_start(out=ctile[:], in_=compd[f, sc * 128:(sc + 1) * 128, :])
                    nc.gpsimd.tensor_tensor(out=pfx[sc][:], in0=pfx[sc][:], in1=ctile[:], op=ADD)
                    idx = small.tile([128, 1], i32, tag="idx8", bufs=8)
                    nc.gpsimd.dma_start(out=idx[:], in_=topk[f, sc * 128:(sc + 1) * 128, None])
                    idxs.append(idx)
                for p0 in range(0, NSC, 2):
                    pair = range(p0, min(p0 + 2, NSC))
                    pg = {}
                    for sc in pair:
                        grow = small.tile([128, D], f32, tag="grow4", bufs=2)
                        nc.gpsimd.indirect_dma_start(
                            out=grow[:], out_offset=None, in_=opv,
                            in_offset=bass.IndirectOffsetOnAxis(ap=idxs[sc][:, :1], axis=1),
                            element_offset=f * S * D)
                        pg[sc] = grow
                    for sc in pair:
                        nc.gpsimd.tensor_tensor(out=pg[sc][:], in0=pg[sc][:], in1=pfx[sc][:], op=ADD)
                    for sc in pair:
                        nc.gpsimd.indirect_dma_start(
                            out=opv, out_offset=bass.IndirectOffsetOnAxis(ap=idxs[sc][:, :1], axis=1),
                            in_=pg[sc][:], in_offset=None,
                            element_offset=f * S * D)

    nc.compile()
    return nc


def _host_prep(hidden_states, topk_idx, weights):
    """Build the 8 per-core input maps (pure data movement + dtype casts)."""
    bf = ml_dtypes.bfloat16
    hs = np.asarray(hidden_states, np.float32)
    (wq, wk, wv, wo, wcq, wck, wcv, wco, wprim,
     b_prim, b_cq, b_ck, b_cv, b_co, b_o) = weights

    wq_s = (wq * SCALE).astype(bf)
    wcq_s = (wcq * SCALE).astype(bf)
    b_cq_s = (b_cq * SCALE).astype(np.float32)
    wcast = {
        "wq": wq_s, "wk": wk.astype(bf), "wv": wv.astype(bf), "wo": wo.astype(bf),
        "wcq": wcq_s, "wck": wck.astype(bf), "wcv": wcv.astype(bf),
        "wco": wco.astype(bf), "wprim": wprim.astype(bf),
    }
    shared = dict(wcast)
    shared["b_prim"] = b_prim.astype(np.float32)
    shared["b_cq"] = b_cq_s
    shared["b_ck"] = b_ck.astype(np.float32)
    shared["brep_cv"] = np.broadcast_to(b_cv.astype(bf), (128, D)).copy()
    shared["brep_co"] = np.broadcast_to(b_co.astype(bf), (128, D)).copy()
    shared["brep_o"] = np.broadcast_to(b_o.astype(bf), (128, D)).copy()

    in_maps = []
    for c in range(NCORES):
        f0, f1 = 2 * c, 2 * c + 1
        p0 = max(f0 - 1, 0)
        m = dict(shared)
        m["hsT"] = np.ascontiguousarray(
            hs[[p0, f0, f1]].transpose(0, 2, 1)).astype(bf)
        g1 = np.stack([hs[f0][topk_idx[f0]], hs[f1][topk_idx[f1]]])
        g2 = np.stack([hs[p0][topk_idx[f0]], hs[f0][topk_idx[f1]]])
        m["g1t"] = np.ascontiguousarray(g1.transpose(0, 2, 1)).astype(bf)
        m["g2t"] = np.ascontiguousarray(g2.transpose(0, 2, 1)).astype(bf)
        m["topk"] = np.ascontiguousarray(topk_idx[[f0, f1]]).astype(np.int32)
        wm = np.zeros((128, NCORES), np.float32)
        wm[:, :c] = 1.0
        m["wmask"] = wm
        in_maps.append(m)
    return in_maps


def kernel(hidden_states, primary_lin_w, primary_lin_b,
           comp_q_w, comp_q_b, comp_k_w, comp_k_b, comp_v_w, comp_v_b,
           comp_out_w, comp_out_b,
           to_q_w, to_k_w, to_v_w, to_out_w, to_out_b,
           video_length):
    import jax
    import jax.numpy as jnp
    from concourse.bass_utils import run_bass_kernel_spmd

    f = int(video_length)
    assert f == F and hidden_states.shape == (F, S, D)

    # Top-k selection: replicate the reference's exact eager-jax computation
    # (selection order is rounding-critical; must match bitwise).
    hs_j = jnp.asarray(np.asarray(hidden_states, np.float32))
    pre = jnp.concatenate([hs_j[:1], hs_j[:-1]], axis=0)
    diff_score = jnp.abs(hs_j - pre).mean(axis=-1)
    _, topk_idx = jax.lax.top_k(diff_score, max(64, S // 2))
    topk_idx = np.asarray(topk_idx)

    weights = tuple(np.asarray(w, np.float32) for w in (
        to_q_w, to_k_w, to_v_w, to_out_w, comp_q_w, comp_k_w, comp_v_w,
        comp_out_w, primary_lin_w, primary_lin_b, comp_q_b, comp_k_b,
        comp_v_b, comp_out_b, to_out_b))
    in_maps = _host_prep(hidden_states, topk_idx, weights)

    if "nc" not in _cache:
        _cache["nc"] = _build_nc()
    res = run_bass_kernel_spmd(_cache["nc"], in_maps, list(range(NCORES)))
    out = np.concatenate([res.results[c]["out"] for c in range(NCORES)], axis=0)
    return np.ascontiguousarray(out.astype(np.float32))


# revision 33
# speedup vs baseline: 1.1982x; 1.0131x over previous
"""Trainium2 Bass kernel for nn_BasicTransformerBlock_50208167690869.

Sparse-attention transformer block, sharded data-parallel over the 16-frame
axis across 8 NeuronCores (2 frames/core, 1-frame halo for the per-frame
shift). The cross-frame cumsum of the compression branch is realized with an
on-device AllGather of each core's local comp sum plus a masked prefix
reduction. Top-k selection is computed on host with the exact same jax ops
as the reference (bitwise-identical selection; the ranking is
rounding-critical), everything else runs on device.

Self-contained: hardcodes shapes from the problem spec.
"""
import sys

for _p in ("/opt/trn_rl_repo",):
    if _p not in sys.path:
        sys.path.append(_p)

import numpy as np
import ml_dtypes

HEADS = 8
DH = 80
D = 640
F = 16
S = 1024
K = 512
NCORES = 8
FPC = F // NCORES  # frames per core
SCALE = DH ** -0.5
NKC = D // 128     # 5 contraction chunks
NTC = S // 128     # 8 token chunks
NSC = K // 128     # 4 slot chunks

_cache = {}


def _apply_drain_patch():
    """This walrus build rejects >1 sync-wait on CTRL instructions; split the
    TileContext tail-drain waits across single-wait nops."""
    import concourse.tile as tile
    from concourse.vector_clock import ScopedClock, VectorClock

    if getattr(tile.TileContext, "_drain_patched", False):
        return

    def _patched(self, tick_clock, wait_clock):
        nc = self.nc
        gc = tick_clock.global_clock
        n = len(gc)
        for p in range(n):
            t = gc[p]
            if t == 0:
                continue
            vc = VectorClock([t if i == p else 0 for i in range(n)])
            nop_inst = nc.sync.nop()
            wait_clock.add_sem_waits(nop_inst.ins, ScopedClock({None: vc}))
        nc.sync.drain()
        nc.all_engine_barrier()
        assert self.sems is not None
        popped = nc._tile_sem_poison_stack.pop()
        assert popped is self._sem_poison
        nc.clear_and_free_semaphores(list(self.sems.allocated().values()))
        nc.all_engine_barrier()

    tile.TileContext._drain_and_barrier = _patched
    tile.TileContext._drain_patched = True


def _build_nc():
    import concourse.bass as bass
    import concourse.bacc as bacc
    import concourse.mybir as mybir
    import concourse.tile as tile
    from concourse.masks import make_identity

    _apply_drain_patch()

    bf16 = mybir.dt.bfloat16
    f32 = mybir.dt.float32
    i32 = mybir.dt.int32
    ADD = mybir.AluOpType.add
    SUB = mybir.AluOpType.subtract
    MUL = mybir.AluOpType.mult

    nc = bacc.Bacc("TRN2", target_bir_lowering=False, num_devices=NCORES)

    # ---- params ----
    hsT = nc.declare_dram_parameter("hsT", [3, D, S], bf16, isOutput=False)
    g1t = nc.declare_dram_parameter("g1t", [FPC, D, K], bf16, isOutput=False)
    g2t = nc.declare_dram_parameter("g2t", [FPC, D, K], bf16, isOutput=False)
    topk = nc.declare_dram_parameter("topk", [FPC, K], i32, isOutput=False)
    wnames = ["wq", "wk", "wv", "wo", "wcq", "wck", "wcv", "wco", "wprim"]
    wd = {n: nc.declare_dram_parameter(n, [D, D], bf16, isOutput=False) for n in wnames}
    b_prim = nc.declare_dram_parameter("b_prim", [D], f32, isOutput=False)
    b_cq = nc.declare_dram_parameter("b_cq", [D], f32, isOutput=False)
    b_ck = nc.declare_dram_parameter("b_ck", [D], f32, isOutput=False)
    brep_cv = nc.declare_dram_parameter("brep_cv", [128, D], bf16, isOutput=False)
    brep_co = nc.declare_dram_parameter("brep_co", [128, D], bf16, isOutput=False)
    brep_o = nc.declare_dram_parameter("brep_o", [128, D], bf16, isOutput=False)
    wmask = nc.declare_dram_parameter("wmask", [128, NCORES], f32, isOutput=False)
    out_ps = [nc.declare_dram_parameter("out0", [S, D], f32, isOutput=True),
              nc.declare_dram_parameter("out1", [S, D], f32, isOutput=True)]

    with tile.TileContext(nc) as tc:
        with tc.tile_pool(name="wp", bufs=1) as wp, \
             tc.tile_pool(name="pfm", bufs=1) as pfm, \
             tc.tile_pool(name="small", bufs=2) as small, \
             tc.tile_pool(name="pexp", bufs=4) as pexp, \
             tc.tile_pool(name="psA", bufs=4, space="PSUM") as psA, \
             tc.tile_pool(name="psO", bufs=3, space="PSUM") as psO, \
             tc.tile_pool(name="psT", bufs=1, space="PSUM") as psT, \
             tc.tile_pool(name="dram", bufs=1, space="DRAM") as dram:

            # ---- persistent constants ----
            ident = wp.tile([128, 128], bf16, tag="ident")
            make_identity(nc, ident[:])
            ones_row = wp.tile([1, 128], bf16, tag="ones")
            nc.gpsimd.memset(ones_row[:], 1.0)

            bprim_sb = wp.tile([128, NKC], f32, tag="bprim")
            for kc in range(NKC):
                nc.sync.dma_start(out=bprim_sb[:, kc:kc + 1], in_=b_prim[kc * 128:(kc + 1) * 128, None])
            bcq_sb = wp.tile([DH, HEADS], f32, tag="bcq")
            bck_sb = wp.tile([DH, HEADS], f32, tag="bck")
            for h in range(HEADS):
                nc.sync.dma_start(out=bcq_sb[:, h:h + 1], in_=b_cq[DH * h:DH * (h + 1), None])
                nc.sync.dma_start(out=bck_sb[:, h:h + 1], in_=b_ck[DH * h:DH * (h + 1), None])
            brepcv_sb = wp.tile([128, D], bf16, tag="brepcv")
            nc.sync.dma_start(out=brepcv_sb[:], in_=brep_cv[:])
            brepco_sb = wp.tile([128, D], bf16, tag="brepco")
            nc.sync.dma_start(out=brepco_sb[:], in_=brep_co[:])
            brepo_sb = wp.tile([128, D], bf16, tag="brepo")
            nc.sync.dma_start(out=brepo_sb[:], in_=brep_o[:])
            wmask_sb = wp.tile([128, NCORES], f32, tag="wmask")
            nc.sync.dma_start(out=wmask_sb[:], in_=wmask[:])

            g1_pre = pfm.tile([128, NKC, K], bf16, tag="g1", name="g1_pre")
            g2_pre = pfm.tile([128, NKC, K], bf16, tag="g2", name="g2_pre")
            for kc in range(NKC):
                nc.gpsimd.dma_start(out=g1_pre[:, kc, :], in_=g1t[0, kc * 128:(kc + 1) * 128, :])
                nc.gpsimd.dma_start(out=g2_pre[:, kc, :], in_=g2t[0, kc * 128:(kc + 1) * 128, :])

            wsb = {}
            _weng = [nc.scalar, nc.sync, nc.gpsimd]
            for wi, n in enumerate(["wprim", "wcq", "wck", "wcv", "wq", "wk", "wv"]):
                t = wp.tile([128, NKC, D], bf16, tag=f"w_{n}")
                for kc in range(NKC):
                    _weng[(wi + kc) % 3].dma_start(out=t[:, kc, :], in_=wd[n][kc * 128:(kc + 1) * 128, :])
                wsb[n] = t
            for wi, n in enumerate(["wco", "wo"]):
                t = wp.tile([DH, HEADS, D], bf16, tag=f"w_{n}")
                for h in range(HEADS):
                    _weng[(wi + h) % 3].dma_start(out=t[:, h, :], in_=wd[n][DH * h:DH * (h + 1), :])
                wsb[n] = t

            def load_hs(slot, nm):
                t = pfm.tile([128, NKC, S], bf16, tag="hs", name=nm, bufs=2)
                _he = [nc.scalar, nc.sync]
                for kc in range(NKC):
                    _he[kc % 2].dma_start(out=t[:, kc, :], in_=hsT[slot, kc * 128:(kc + 1) * 128, :])
                return t

            # ---- DRAM scratch ----
            compd = dram.tile([FPC, K, D], f32, tag="compd")
            cc_in = dram.tile([K, D], f32, tag="cc_in")
            cc_out = dram.tile([NCORES, K, D], f32, tag="cc_out")

            # ---- helpers ----
            def proj_fm_head(w, bhead, src, N, out_tag):
                """Per-head FM projection: out [DH, HEADS, N] bf16 = W.T @ src (+bias)."""
                o = pfm.tile([DH, HEADS, N], bf16, tag=out_tag)
                for h in range(HEADS):
                    for n0 in range(0, N, 512):
                        nn = min(512, N - n0)
                        ps = psA.tile([128, 512], f32, tag="psA")
                        for kc in range(NKC):
                            nc.tensor.matmul(
                                out=ps[0:DH, 0:nn],
                                lhsT=w[:, kc, DH * h:DH * (h + 1)],
                                rhs=src[:, kc, n0:n0 + nn],
                                start=(kc == 0), stop=(kc == NKC - 1))
                        if bhead is not None:
                            nc.vector.tensor_scalar(
                                o[:, h, n0:n0 + nn], ps[0:DH, 0:nn],
                                bhead[:, h:h + 1], None, op0=ADD)
                        else:
                            nc.scalar.copy(o[:, h, n0:n0 + nn], ps[0:DH, 0:nn])
                return o

            def proj_tm_v(src, w, brep, frame_tag):
                """TM projection with ones column: list of 8 tiles [128, HEADS, DH+1]."""
                tiles = []
                for tcid in range(NTC):
                    psL = psA.tile([128, 512], f32, tag="psA")
                    psR = psA.tile([128, 512], f32, tag="psA")
                    for kc in range(NKC):
                        nc.tensor.matmul(
                            out=psL[:, 0:512],
                            lhsT=src[:, kc, tcid * 128:(tcid + 1) * 128],
                            rhs=w[:, kc, 0:512],
                            start=(kc == 0), stop=(kc == NKC - 1))
                    for kc in range(NKC):
                        nc.tensor.matmul(
                            out=psR[:, 0:128],
                            lhsT=src[:, kc, tcid * 128:(tcid + 1) * 128],
                            rhs=w[:, kc, 512:640],
                            start=(kc == 0), stop=(kc == NKC - 1))
                    v = pfm.tile([128, HEADS, 97], bf16, tag=f"{frame_tag}{tcid}")
                    nc.vector.memset(v[:, :, DH:96], 0.0)
                    nc.vector.memset(v[:, :, 96:97], 1.0)

                    def _vcopy(hh, d0, d1, srcap, c0):
                        if brep is not None:
                            nc.vector.tensor_tensor(
                                out=v[:, hh, d0:d1], in0=srcap,
                                in1=brep[:, c0:c0 + (d1 - d0)], op=ADD)
                        else:
                            nc.vector.tensor_copy(v[:, hh, d0:d1], srcap)

                    for h in range(6):
                        _vcopy(h, 0, DH, psL[:, DH * h:DH * (h + 1)], DH * h)
                    _vcopy(6, 0, 32, psL[:, 480:512], 480)
                    _vcopy(6, 32, DH, psR[:, 0:48], 512)
                    _vcopy(7, 0, DH, psR[:, 48:128], 560)
                    tiles.append(v)
                return tiles

            def proj_head_into(w, bhead, src, N, otile, h):
                for n0 in range(0, N, 512):
                    nn = min(512, N - n0)
                    ps = psA.tile([128, 512], f32, tag="psA")
                    for kc in range(NKC):
                        nc.tensor.matmul(
                            out=ps[0:DH, 0:nn],
                            lhsT=w[:, kc, DH * h:DH * (h + 1)],
                            rhs=src[:, kc, n0:n0 + nn],
                            start=(kc == 0), stop=(kc == NKC - 1))
                    if bhead is not None:
                        nc.vector.tensor_scalar(
                            otile[:, h, n0:n0 + nn], ps[0:DH, 0:nn],
                            bhead[:, h:h + 1], None, op0=ADD)
                    else:
                        nc.scalar.copy(otile[:, h, n0:n0 + nn], ps[0:DH, 0:nn])

            def fused_branch(qw, qb, ksrcw, kb, qsrc, ksrc, vts, NQ,
                             qtag, ktag, out_tag):
                """Per-head pipelined projection + attention: head h+1's q/k
                projections are emitted before head h's attention units so the
                PE->DVE/ACT copy handoff is hidden behind matmul work."""
                qT = pfm.tile([DH, HEADS, NQ], bf16, tag=qtag, name=qtag + "t")
                kT = pfm.tile([DH, HEADS, S], bf16, tag=ktag, name=ktag + "t")
                o = pfm.tile([DH, HEADS, NQ], bf16, tag=out_tag, name=out_tag + "t")
                pending = []

                def proj_head(h):
                    proj_head_into(qw, qb, qsrc, NQ, qT, h)
                    proj_head_into(ksrcw, kb, ksrc, S, kT, h)

                def flush_norm():
                    if not pending:
                        return
                    po, h, n0, nn = pending.pop(0)
                    drow_bf = small.tile([1, 512], bf16, tag="drowbf")
                    nc.scalar.copy(out=drow_bf[:, 0:nn], in_=po[96:97, 0:nn])
                    prep = psA.tile([128, 512], f32, tag="psA")
                    nc.tensor.matmul(
                        out=prep[:, 0:nn], lhsT=ones_row[:],
                        rhs=drow_bf[:, 0:nn], start=True, stop=True)
                    recip_sb = pexp.tile([128, 512], f32, tag="recipsb", bufs=2)
                    nc.vector.reciprocal(out=recip_sb[0:DH, 0:nn], in_=prep[0:DH, 0:nn])
                    nc.vector.tensor_tensor(
                        out=o[:, h, n0:n0 + nn], in0=po[0:DH, 0:nn],
                        in1=recip_sb[0:DH, 0:nn], op=MUL)

                proj_head(0)
                for h in range(HEADS):
                    if h + 1 < HEADS:
                        proj_head(h + 1)
                    for n0 in range(0, NQ, 512):
                        nn = min(512, NQ - n0)
                        po = psO.tile([128, 512], f32, tag="psO")
                        for tcid in range(NTC):
                            ps = psA.tile([128, 512], f32, tag="psA")
                            nc.tensor.matmul(
                                out=ps[:, 0:nn],
                                lhsT=kT[:, h, tcid * 128:(tcid + 1) * 128],
                                rhs=qT[:, h, n0:n0 + nn],
                                start=True, stop=True)
                            ex = pexp.tile([128, 512], bf16, tag="exp", bufs=3)
                            nc.scalar.activation(
                                out=ex[:, 0:nn], in_=ps[:, 0:nn],
                                func=mybir.ActivationFunctionType.Exp)
                            nc.tensor.matmul(
                                out=po[0:97, 0:nn],
                                lhsT=vts[tcid][:, h, 0:97],
                                rhs=ex[:, 0:nn],
                                start=(tcid == 0), stop=(tcid == NTC - 1))
                        pending.append((po, h, n0, nn))
                        if len(pending) > 1:
                            flush_norm()
                while pending:
                    flush_norm()
                return o

            def attention(qT, kT, vts, NQ, out_tag):
                """FM attention: O^T [DH, HEADS, NQ] bf16, softmax over S keys."""
                o = pfm.tile([DH, HEADS, NQ], bf16, tag=out_tag)
                pending = []

                def flush_norm():
                    if not pending:
                        return
                    po, h, n0, nn = pending.pop(0)
                    drow_bf = small.tile([1, 512], bf16, tag="drowbf")
                    nc.scalar.copy(out=drow_bf[:, 0:nn], in_=po[96:97, 0:nn])
                    prep = psA.tile([128, 512], f32, tag="psA")
                    nc.tensor.matmul(
                        out=prep[:, 0:nn], lhsT=ones_row[:],
                        rhs=drow_bf[:, 0:nn], start=True, stop=True)
                    recip_sb = pexp.tile([128, 512], f32, tag="recipsb", bufs=2)
                    nc.vector.reciprocal(out=recip_sb[0:DH, 0:nn], in_=prep[0:DH, 0:nn])
                    nc.vector.tensor_tensor(
                        out=o[:, h, n0:n0 + nn], in0=po[0:DH, 0:nn],
                        in1=recip_sb[0:DH, 0:nn], op=MUL)

                for h in range(HEADS):
                    for n0 in range(0, NQ, 512):
                        nn = min(512, NQ - n0)
                        po = psO.tile([128, 512], f32, tag="psO")
                        for tcid in range(NTC):
                            ps = psA.tile([128, 512], f32, tag="psA")
                            nc.tensor.matmul(
                                out=ps[:, 0:nn],
                                lhsT=kT[:, h, tcid * 128:(tcid + 1) * 128],
                                rhs=qT[:, h, n0:n0 + nn],
                                start=True, stop=True)
                            ex = pexp.tile([128, 512], bf16, tag="exp", bufs=3)
                            nc.scalar.activation(
                                out=ex[:, 0:nn], in_=ps[:, 0:nn],
                                func=mybir.ActivationFunctionType.Exp)
                            nc.tensor.matmul(
                                out=po[0:97, 0:nn],
                                lhsT=vts[tcid][:, h, 0:97],
                                rhs=ex[:, 0:nn],
                                start=(tcid == 0), stop=(tcid == NTC - 1))
                        pending.append((po, h, n0, nn))
                        if len(pending) > 1:
                            flush_norm()
                while pending:
                    flush_norm()
                return o

            # ================= COMP BRANCH (both frames first) =================
            ssum = [pfm.tile([128, D], f32, tag=f"ssum{sc}", name=f"ssum{sc}") for sc in range(NSC)]
            hs_tiles = []
            for f in range(FPC):
                hs_prev = load_hs(f, f"hsc{f}")
                hs_tiles.append(hs_prev)
                if f == 0:
                    g1, g2 = g1_pre, g2_pre
                else:
                    g1 = pfm.tile([128, NKC, K], bf16, tag="g1")
                    g2 = pfm.tile([128, NKC, K], bf16, tag="g2")
                    for kc in range(NKC):
                        nc.sync.dma_start(out=g1[:, kc, :], in_=g1t[f, kc * 128:(kc + 1) * 128, :])
                        nc.sync.dma_start(out=g2[:, kc, :], in_=g2t[f, kc * 128:(kc + 1) * 128, :])
                gd = pfm.tile([128, NKC, K], bf16, tag="zt")
                nc.vector.tensor_tensor(out=gd[:], in0=g1[:], in1=g2[:], op=SUB)

                # primary_hs (FM, chunk-aligned): prim = gd @ Wprim + b + g1
                prim = pfm.tile([128, NKC, K], bf16, tag="prim")
                for mc in range(NKC):
                    ps = psA.tile([128, 512], f32, tag="psA")
                    for kc in range(NKC):
                        nc.tensor.matmul(
                            out=ps[:], lhsT=wsb["wprim"][:, kc, mc * 128:(mc + 1) * 128],
                            rhs=gd[:, kc, :], start=(kc == 0), stop=(kc == NKC - 1))
                    nc.vector.tensor_scalar(
                        prim[:, mc, :], ps[:], bprim_sb[:, mc:mc + 1], None, op0=ADD)
                    nc.vector.tensor_tensor(
                        out=prim[:, mc, :], in0=prim[:, mc, :], in1=g1[:, mc, :], op=ADD)

                vC = proj_tm_v(hs_prev, wsb["wcv"], brepcv_sb, "v")
                ocT = fused_branch(wsb["wcq"], bcq_sb, wsb["wck"], bck_sb,
                                   prim, hs_prev, vC, K, "qT", "kT", "oT")

                # comp_out projection (TM) + bias; spill to DRAM + local sum
                for sc in range(NSC):
                    psL = psA.tile([128, 512], f32, tag="psA")
                    psR = psA.tile([128, 512], f32, tag="psA")
                    for h in range(HEADS):
                        nc.tensor.matmul(
                            out=psL[:, 0:512],
                            lhsT=ocT[:, h, sc * 128:(sc + 1) * 128],
                            rhs=wsb["wco"][:, h, 0:512],
                            start=(h == 0), stop=(h == HEADS - 1))
                    for h in range(HEADS):
                        nc.tensor.matmul(
                            out=psR[:, 0:128],
                            lhsT=ocT[:, h, sc * 128:(sc + 1) * 128],
                            rhs=wsb["wco"][:, h, 512:640],
                            start=(h == 0), stop=(h == HEADS - 1))
                    csb = small.tile([128, D], f32, tag="wk640")
                    nc.vector.tensor_tensor(out=csb[:, 0:512], in0=psL[:, 0:512],
                                            in1=brepco_sb[:, 0:512], op=ADD)
                    nc.vector.tensor_tensor(out=csb[:, 512:640], in0=psR[:, 0:128],
                                            in1=brepco_sb[:, 512:640], op=ADD)
                    nc.sync.dma_start(out=compd[f, sc * 128:(sc + 1) * 128, :], in_=csb[:])
                    if f == 0:
                        nc.vector.tensor_copy(ssum[sc][:], csb[:])
                    else:
                        nc.vector.tensor_tensor(out=ssum[sc][:], in0=ssum[sc][:], in1=csb[:], op=ADD)

            for sc in range(NSC):
                nc.sync.dma_start(out=cc_in[sc * 128:(sc + 1) * 128, :], in_=ssum[sc][:])
            nc.gpsimd.collective_compute(
                "AllGather", mybir.AluOpType.bypass,
                replica_groups=[list(range(NCORES))],
                ins=[cc_in.opt()], outs=[cc_out.opt()])

            # masked prefix, entirely on the gpsimd queue (inherently ordered
            # after the collective; keeps the PE/DVE/sync queues unblocked)
            pfx = [pfm.tile([128, D], f32, tag=f"ssum{sc}", name=f"pfx{sc}")
                   for sc in range(NSC)]
            for sc in range(NSC):
                nc.gpsimd.memset(pfx[sc][:], 0.0)
                for cc in range(NCORES):
                    tmp = small.tile([128, D], f32, tag="wk640b")
                    nc.gpsimd.dma_start(out=tmp[:], in_=cc_out[cc, sc * 128:(sc + 1) * 128, :])
                    nc.gpsimd.tensor_tensor(
                        out=tmp[:], in0=tmp[:],
                        in1=wmask_sb[:, cc:cc + 1].to_broadcast([128, D]), op=MUL)
                    nc.gpsimd.tensor_tensor(out=pfx[sc][:], in0=pfx[sc][:], in1=tmp[:], op=ADD)

            # ================= MAIN BRANCH =================
            for f in range(FPC):
                if f == 0:
                    hs_prev = hs_tiles[0]
                    hs_own = hs_tiles[1]
                else:
                    hs_prev = hs_tiles[1]
                    hs_own = load_hs(2, "hsm2")
                v2 = proj_tm_v(hs_prev, wsb["wv"], None, "v")
                o2T = fused_branch(wsb["wq"], None, wsb["wk"], None,
                                   hs_own, hs_prev, v2, S, "qT", "kT", "oT")

                # out projection -> Z^T bf16
                zt = pfm.tile([128, NKC, S], bf16, tag="zt")
                for mc in range(NKC):
                    for n0 in range(0, S, 512):
                        ps = psA.tile([128, 512], f32, tag="psA")
                        for h in range(HEADS):
                            nc.tensor.matmul(
                                out=ps[:],
                                lhsT=wsb["wo"][:, h, mc * 128:(mc + 1) * 128],
                                rhs=o2T[:, h, n0:n0 + 512],
                                start=(h == 0), stop=(h == HEADS - 1))
                        nc.scalar.copy(out=zt[:, mc, n0:n0 + 512], in_=ps[:])

                # transpose to TM, add bias, store
                for tcid in range(NTC):
                    pt = psT.tile([128, D], bf16, tag="psT")
                    for mc in range(NKC):
                        nc.tensor.transpose(
                            out=pt[:, mc * 128:(mc + 1) * 128],
                            in_=zt[:, mc, tcid * 128:(tcid + 1) * 128],
                            identity=ident[:])
                    zfin = small.tile([128, D], f32, tag="wk640")
                    nc.vector.tensor_tensor(out=zfin[:], in0=pt[:], in1=brepo_sb[:], op=ADD)
                    nc.sync.dma_start(out=out_ps[f][tcid * 128:(tcid + 1) * 128, :], in_=zfin[:])



            # cumsum + scatter-add RMW, entirely on the gpsimd queue;
            # phased per frame so the 4 indirect gathers pipeline their transfers
            for f in range(FPC):
                opv = out_ps[f][:]
                idxs = []
                for sc in range(NSC):
                    ctile = small.tile([128, D], f32, tag="wk640b")
                    nc.gpsimd.dma_start(out=ctile[:], in_=compd[f, sc * 128:(sc + 1) * 128, :])
                    nc.gpsimd.tensor_tensor(out=pfx[sc][:], in0=pfx[sc][:], in1=ctile[:], op=ADD)
                    idx = small.tile([128, 1], i32, tag="idx8", bufs=8)
                    nc.gpsimd.dma_start(out=idx[:], in_=topk[f, sc * 128:(sc + 1) * 128, None])
                    idxs.append(idx)
                for p0 in range(0, NSC, 2):
                    pair = range(p0, min(p0 + 2, NSC))
                    pg = {}
                    for sc in pair:
                        grow = small.tile([128, D], f32, tag="grow4", bufs=2)
                        nc.gpsimd.indirect_dma_start(
                            out=grow[:], out_offset=None, in_=opv,
                            in_offset=bass.IndirectOffsetOnAxis(ap=idxs[sc][:, :1], axis=0))
                        pg[sc] = grow
                    for sc in pair:
                        nc.gpsimd.tensor_tensor(out=pg[sc][:], in0=pg[sc][:], in1=pfx[sc][:], op=ADD)
                    for sc in pair:
                        nc.gpsimd.indirect_dma_start(
                            out=opv, out_offset=bass.IndirectOffsetOnAxis(ap=idxs[sc][:, :1], axis=0),
                            in_=pg[sc][:], in_offset=None)

    nc.compile()
    return nc


def _host_prep(hidden_states, topk_idx, weights):
    """Build the 8 per-core input maps (pure data movement + dtype casts)."""
    bf = ml_dtypes.bfloat16
    hs = np.asarray(hidden_states, np.float32)
    (wq, wk, wv, wo, wcq, wck, wcv, wco, wprim,
     b_prim, b_cq, b_ck, b_cv, b_co, b_o) = weights

    wq_s = (wq * SCALE).astype(bf)
    wcq_s = (wcq * SCALE).astype(bf)
    b_cq_s = (b_cq * SCALE).astype(np.float32)
    wcast = {
        "wq": wq_s, "wk": wk.astype(bf), "wv": wv.astype(bf), "wo": wo.astype(bf),
        "wcq": wcq_s, "wck": wck.astype(bf), "wcv": wcv.astype(bf),
        "wco": wco.astype(bf), "wprim": wprim.astype(bf),
    }
    shared = dict(wcast)
    shared["b_prim"] = b_prim.astype(np.float32)
    shared["b_cq"] = b_cq_s
    shared["b_ck"] = b_ck.astype(np.float32)
    shared["brep_cv"] = np.broadcast_to(b_cv.astype(bf), (128, D)).copy()
    shared["brep_co"] = np.broadcast_to(b_co.astype(bf), (128, D)).copy()
    shared["brep_o"] = np.broadcast_to(b_o.astype(bf), (128, D)).copy()

    in_maps = []
    for c in range(NCORES):
        f0, f1 = 2 * c, 2 * c + 1
        p0 = max(f0 - 1, 0)
        m = dict(shared)
        m["hsT"] = np.ascontiguousarray(
            hs[[p0, f0, f1]].transpose(0, 2, 1)).astype(bf)
        g1 = np.stack([hs[f0][topk_idx[f0]], hs[f1][topk_idx[f1]]])
        g2 = np.stack([hs[p0][topk_idx[f0]], hs[f0][topk_idx[f1]]])
        m["g1t"] = np.ascontiguousarray(g1.transpose(0, 2, 1)).astype(bf)
        m["g2t"] = np.ascontiguousarray(g2.transpose(0, 2, 1)).astype(bf)
        m["topk"] = np.ascontiguousarray(topk_idx[[f0, f1]]).astype(np.int32)
        wm = np.zeros((128, NCORES), np.float32)
        wm[:, :c] = 1.0
        m["wmask"] = wm
        in_maps.append(m)
    return in_maps


def kernel(hidden_states, primary_lin_w, primary_lin_b,
           comp_q_w, comp_q_b, comp_k_w, comp_k_b, comp_v_w, comp_v_b,
           comp_out_w, comp_out_b,
           to_q_w, to_k_w, to_v_w, to_out_w, to_out_b,
           video_length):
    import jax
    import jax.numpy as jnp
    from concourse.bass_utils import run_bass_kernel_spmd

    f = int(video_length)
    assert f == F and hidden_states.shape == (F, S, D)

    # Top-k selection: replicate the reference's exact eager-jax computation
    # (selection order is rounding-critical; must match bitwise).
    hs_j = jnp.asarray(np.asarray(hidden_states, np.float32))
    pre = jnp.concatenate([hs_j[:1], hs_j[:-1]], axis=0)
    diff_score = jnp.abs(hs_j - pre).mean(axis=-1)
    _, topk_idx = jax.lax.top_k(diff_score, max(64, S // 2))
    topk_idx = np.asarray(topk_idx)

    weights = tuple(np.asarray(w, np.float32) for w in (
        to_q_w, to_k_w, to_v_w, to_out_w, comp_q_w, comp_k_w, comp_v_w,
        comp_out_w, primary_lin_w, primary_lin_b, comp_q_b, comp_k_b,
        comp_v_b, comp_out_b, to_out_b))
    in_maps = _host_prep(hidden_states, topk_idx, weights)

    if "nc" not in _cache:
        _cache["nc"] = _build_nc()
    res = run_bass_kernel_spmd(_cache["nc"], in_maps, list(range(NCORES)))
    out = np.concatenate(
        [np.stack([res.results[c]["out0"], res.results[c]["out1"]])
         for c in range(NCORES)], axis=0)
    return np.ascontiguousarray(out.astype(np.float32))


# revision 34
# speedup vs baseline: 1.2052x; 1.0058x over previous
"""Trainium2 Bass kernel for nn_BasicTransformerBlock_50208167690869.

Sparse-attention transformer block, sharded data-parallel over the 16-frame
axis across 8 NeuronCores (2 frames/core, 1-frame halo for the per-frame
shift). The cross-frame cumsum of the compression branch is realized with an
on-device AllGather of each core's local comp sum plus a masked prefix
reduction. Top-k selection is computed on host with the exact same jax ops
as the reference (bitwise-identical selection; the ranking is
rounding-critical), everything else runs on device.

Self-contained: hardcodes shapes from the problem spec.
"""
import sys

for _p in ("/opt/trn_rl_repo",):
    if _p not in sys.path:
        sys.path.append(_p)

import numpy as np
import ml_dtypes

HEADS = 8
DH = 80
D = 640
F = 16
S = 1024
K = 512
NCORES = 8
FPC = F // NCORES  # frames per core
SCALE = DH ** -0.5
NKC = D // 128     # 5 contraction chunks
NTC = S // 128     # 8 token chunks
NSC = K // 128     # 4 slot chunks

_cache = {}


def _apply_drain_patch():
    """This walrus build rejects >1 sync-wait on CTRL instructions; split the
    TileContext tail-drain waits across single-wait nops."""
    import concourse.tile as tile
    from concourse.vector_clock import ScopedClock, VectorClock

    if getattr(tile.TileContext, "_drain_patched", False):
        return

    def _patched(self, tick_clock, wait_clock):
        nc = self.nc
        gc = tick_clock.global_clock
        n = len(gc)
        for p in range(n):
            t = gc[p]
            if t == 0:
                continue
            vc = VectorClock([t if i == p else 0 for i in range(n)])
            nop_inst = nc.sync.nop()
            wait_clock.add_sem_waits(nop_inst.ins, ScopedClock({None: vc}))
        nc.sync.drain()
        nc.all_engine_barrier()
        assert self.sems is not None
        popped = nc._tile_sem_poison_stack.pop()
        assert popped is self._sem_poison
        nc.clear_and_free_semaphores(list(self.sems.allocated().values()))
        nc.all_engine_barrier()

    tile.TileContext._drain_and_barrier = _patched
    tile.TileContext._drain_patched = True


def _build_nc():
    import concourse.bass as bass
    import concourse.bacc as bacc
    import concourse.mybir as mybir
    import concourse.tile as tile
    from concourse.masks import make_identity

    _apply_drain_patch()

    bf16 = mybir.dt.bfloat16
    f32 = mybir.dt.float32
    i32 = mybir.dt.int32
    ADD = mybir.AluOpType.add
    SUB = mybir.AluOpType.subtract
    MUL = mybir.AluOpType.mult

    nc = bacc.Bacc("TRN2", target_bir_lowering=False, num_devices=NCORES)

    # ---- params ----
    hsT = nc.declare_dram_parameter("hsT", [3, D, S], bf16, isOutput=False)
    g1t = nc.declare_dram_parameter("g1t", [FPC, D, K], bf16, isOutput=False)
    g2t = nc.declare_dram_parameter("g2t", [FPC, D, K], bf16, isOutput=False)
    topk = nc.declare_dram_parameter("topk", [FPC, K], i32, isOutput=False)
    wnames = ["wq", "wk", "wv", "wo", "wcq", "wck", "wcv", "wco", "wprim"]
    wd = {n: nc.declare_dram_parameter(n, [D, D], bf16, isOutput=False) for n in wnames}
    b_prim = nc.declare_dram_parameter("b_prim", [D], f32, isOutput=False)
    b_cq = nc.declare_dram_parameter("b_cq", [D], f32, isOutput=False)
    b_ck = nc.declare_dram_parameter("b_ck", [D], f32, isOutput=False)
    brep_cv = nc.declare_dram_parameter("brep_cv", [128, D], bf16, isOutput=False)
    brep_co = nc.declare_dram_parameter("brep_co", [128, D], bf16, isOutput=False)
    brep_o = nc.declare_dram_parameter("brep_o", [128, D], bf16, isOutput=False)
    wmask = nc.declare_dram_parameter("wmask", [128, NCORES], f32, isOutput=False)
    out_ps = [nc.declare_dram_parameter("out0", [S, D], f32, isOutput=True),
              nc.declare_dram_parameter("out1", [S, D], f32, isOutput=True)]

    with tile.TileContext(nc) as tc:
        with tc.tile_pool(name="wp", bufs=1) as wp, \
             tc.tile_pool(name="pfm", bufs=1) as pfm, \
             tc.tile_pool(name="small", bufs=2) as small, \
             tc.tile_pool(name="pexp", bufs=4) as pexp, \
             tc.tile_pool(name="psA", bufs=4, space="PSUM") as psA, \
             tc.tile_pool(name="psO", bufs=3, space="PSUM") as psO, \
             tc.tile_pool(name="psT", bufs=1, space="PSUM") as psT, \
             tc.tile_pool(name="dram", bufs=1, space="DRAM") as dram:

            # ---- persistent constants ----
            ident = wp.tile([128, 128], bf16, tag="ident")
            make_identity(nc, ident[:])
            ones_row = wp.tile([1, 128], bf16, tag="ones")
            nc.gpsimd.memset(ones_row[:], 1.0)

            bprim_sb = wp.tile([128, NKC], f32, tag="bprim")
            for kc in range(NKC):
                nc.sync.dma_start(out=bprim_sb[:, kc:kc + 1], in_=b_prim[kc * 128:(kc + 1) * 128, None])
            bcq_sb = wp.tile([DH, HEADS], f32, tag="bcq")
            bck_sb = wp.tile([DH, HEADS], f32, tag="bck")
            for h in range(HEADS):
                nc.sync.dma_start(out=bcq_sb[:, h:h + 1], in_=b_cq[DH * h:DH * (h + 1), None])
                nc.sync.dma_start(out=bck_sb[:, h:h + 1], in_=b_ck[DH * h:DH * (h + 1), None])
            brepcv_sb = wp.tile([128, D], bf16, tag="brepcv")
            nc.sync.dma_start(out=brepcv_sb[:], in_=brep_cv[:])
            brepco_sb = wp.tile([128, D], bf16, tag="brepco")
            nc.sync.dma_start(out=brepco_sb[:], in_=brep_co[:])
            brepo_sb = wp.tile([128, D], bf16, tag="brepo")
            nc.sync.dma_start(out=brepo_sb[:], in_=brep_o[:])
            wmask_sb = wp.tile([128, NCORES], f32, tag="wmask")
            nc.sync.dma_start(out=wmask_sb[:], in_=wmask[:])

            g1_pre = pfm.tile([128, NKC, K], bf16, tag="g1", name="g1_pre")
            g2_pre = pfm.tile([128, NKC, K], bf16, tag="g2", name="g2_pre")
            for kc in range(NKC):
                nc.gpsimd.dma_start(out=g1_pre[:, kc, :], in_=g1t[0, kc * 128:(kc + 1) * 128, :])
                nc.gpsimd.dma_start(out=g2_pre[:, kc, :], in_=g2t[0, kc * 128:(kc + 1) * 128, :])

            wsb = {}
            _weng = [nc.scalar, nc.sync, nc.gpsimd]
            for wi, n in enumerate(["wprim", "wcq", "wck", "wcv", "wq", "wk", "wv"]):
                t = wp.tile([128, NKC, D], bf16, tag=f"w_{n}")
                for kc in range(NKC):
                    if n == "wprim":
                        e = [nc.scalar, nc.sync][kc % 2]
                    else:
                        e = _weng[(wi + kc) % 3]
                    e.dma_start(out=t[:, kc, :], in_=wd[n][kc * 128:(kc + 1) * 128, :])
                wsb[n] = t
            for wi, n in enumerate(["wco", "wo"]):
                t = wp.tile([DH, HEADS, D], bf16, tag=f"w_{n}")
                for h in range(HEADS):
                    _weng[(wi + h) % 3].dma_start(out=t[:, h, :], in_=wd[n][DH * h:DH * (h + 1), :])
                wsb[n] = t

            def load_hs(slot, nm):
                t = pfm.tile([128, NKC, S], bf16, tag="hs", name=nm, bufs=2)
                _he = [nc.scalar, nc.sync]
                for kc in range(NKC):
                    _he[kc % 2].dma_start(out=t[:, kc, :], in_=hsT[slot, kc * 128:(kc + 1) * 128, :])
                return t

            # ---- DRAM scratch ----
            compd = dram.tile([FPC, K, D], f32, tag="compd")
            cc_in = dram.tile([K, D], f32, tag="cc_in")
            cc_out = dram.tile([NCORES, K, D], f32, tag="cc_out")

            # ---- helpers ----
            def proj_fm_head(w, bhead, src, N, out_tag):
                """Per-head FM projection: out [DH, HEADS, N] bf16 = W.T @ src (+bias)."""
                o = pfm.tile([DH, HEADS, N], bf16, tag=out_tag)
                for h in range(HEADS):
                    for n0 in range(0, N, 512):
                        nn = min(512, N - n0)
                        ps = psA.tile([128, 512], f32, tag="psA")
                        for kc in range(NKC):
                            nc.tensor.matmul(
                                out=ps[0:DH, 0:nn],
                                lhsT=w[:, kc, DH * h:DH * (h + 1)],
                                rhs=src[:, kc, n0:n0 + nn],
                                start=(kc == 0), stop=(kc == NKC - 1))
                        if bhead is not None:
                            nc.vector.tensor_scalar(
                                o[:, h, n0:n0 + nn], ps[0:DH, 0:nn],
                                bhead[:, h:h + 1], None, op0=ADD)
                        else:
                            nc.scalar.copy(o[:, h, n0:n0 + nn], ps[0:DH, 0:nn])
                return o

            def proj_tm_v(src, w, brep, frame_tag):
                """TM projection with ones column: list of 8 tiles [128, HEADS, DH+1]."""
                tiles = []
                for tcid in range(NTC):
                    psL = psA.tile([128, 512], f32, tag="psA")
                    psR = psA.tile([128, 512], f32, tag="psA")
                    for kc in range(NKC):
                        nc.tensor.matmul(
                            out=psL[:, 0:512],
                            lhsT=src[:, kc, tcid * 128:(tcid + 1) * 128],
                            rhs=w[:, kc, 0:512],
                            start=(kc == 0), stop=(kc == NKC - 1))
                    for kc in range(NKC):
                        nc.tensor.matmul(
                            out=psR[:, 0:128],
                            lhsT=src[:, kc, tcid * 128:(tcid + 1) * 128],
                            rhs=w[:, kc, 512:640],
                            start=(kc == 0), stop=(kc == NKC - 1))
                    v = pfm.tile([128, HEADS, 97], bf16, tag=f"{frame_tag}{tcid}")
                    nc.vector.memset(v[:, :, DH:96], 0.0)
                    nc.vector.memset(v[:, :, 96:97], 1.0)

                    def _vcopy(hh, d0, d1, srcap, c0):
                        if brep is not None:
                            nc.vector.tensor_tensor(
                                out=v[:, hh, d0:d1], in0=srcap,
                                in1=brep[:, c0:c0 + (d1 - d0)], op=ADD)
                        else:
                            nc.vector.tensor_copy(v[:, hh, d0:d1], srcap)

                    for h in range(6):
                        _vcopy(h, 0, DH, psL[:, DH * h:DH * (h + 1)], DH * h)
                    _vcopy(6, 0, 32, psL[:, 480:512], 480)
                    _vcopy(6, 32, DH, psR[:, 0:48], 512)
                    _vcopy(7, 0, DH, psR[:, 48:128], 560)
                    tiles.append(v)
                return tiles

            def proj_head_into(w, bhead, src, N, otile, h):
                for n0 in range(0, N, 512):
                    nn = min(512, N - n0)
                    ps = psA.tile([128, 512], f32, tag="psA")
                    for kc in range(NKC):
                        nc.tensor.matmul(
                            out=ps[0:DH, 0:nn],
                            lhsT=w[:, kc, DH * h:DH * (h + 1)],
                            rhs=src[:, kc, n0:n0 + nn],
                            start=(kc == 0), stop=(kc == NKC - 1))
                    if bhead is not None:
                        nc.vector.tensor_scalar(
                            otile[:, h, n0:n0 + nn], ps[0:DH, 0:nn],
                            bhead[:, h:h + 1], None, op0=ADD)
                    else:
                        nc.scalar.copy(otile[:, h, n0:n0 + nn], ps[0:DH, 0:nn])

            def fused_branch(qw, qb, ksrcw, kb, qsrc, ksrc, vts, NQ,
                             qtag, ktag, out_tag):
                """Per-head pipelined projection + attention: head h+1's q/k
                projections are emitted before head h's attention units so the
                PE->DVE/ACT copy handoff is hidden behind matmul work."""
                qT = pfm.tile([DH, HEADS, NQ], bf16, tag=qtag, name=qtag + "t")
                kT = pfm.tile([DH, HEADS, S], bf16, tag=ktag, name=ktag + "t")
                o = pfm.tile([DH, HEADS, NQ], bf16, tag=out_tag, name=out_tag + "t")
                pending = []

                def proj_head(h):
                    proj_head_into(qw, qb, qsrc, NQ, qT, h)
                    proj_head_into(ksrcw, kb, ksrc, S, kT, h)

                def flush_norm():
                    if not pending:
                        return
                    po, h, n0, nn = pending.pop(0)
                    drow_bf = small.tile([1, 512], bf16, tag="drowbf")
                    nc.scalar.copy(out=drow_bf[:, 0:nn], in_=po[96:97, 0:nn])
                    prep = psA.tile([128, 512], f32, tag="psA")
                    nc.tensor.matmul(
                        out=prep[:, 0:nn], lhsT=ones_row[:],
                        rhs=drow_bf[:, 0:nn], start=True, stop=True)
                    recip_sb = pexp.tile([128, 512], f32, tag="recipsb", bufs=2)
                    nc.vector.reciprocal(out=recip_sb[0:DH, 0:nn], in_=prep[0:DH, 0:nn])
                    nc.vector.tensor_tensor(
                        out=o[:, h, n0:n0 + nn], in0=po[0:DH, 0:nn],
                        in1=recip_sb[0:DH, 0:nn], op=MUL)

                proj_head(0)
                for h in range(HEADS):
                    if h + 1 < HEADS:
                        proj_head(h + 1)
                    for n0 in range(0, NQ, 512):
                        nn = min(512, NQ - n0)
                        po = psO.tile([128, 512], f32, tag="psO")
                        for tcid in range(NTC):
                            ps = psA.tile([128, 512], f32, tag="psA")
                            nc.tensor.matmul(
                                out=ps[:, 0:nn],
                                lhsT=kT[:, h, tcid * 128:(tcid + 1) * 128],
                                rhs=qT[:, h, n0:n0 + nn],
                                start=True, stop=True)
                            ex = pexp.tile([128, 512], bf16, tag="exp", bufs=3)
                            nc.scalar.activation(
                                out=ex[:, 0:nn], in_=ps[:, 0:nn],
                                func=mybir.ActivationFunctionType.Exp)
                            nc.tensor.matmul(
                                out=po[0:97, 0:nn],
                                lhsT=vts[tcid][:, h, 0:97],
                                rhs=ex[:, 0:nn],
                                start=(tcid == 0), stop=(tcid == NTC - 1))
                        pending.append((po, h, n0, nn))
                        if len(pending) > 1:
                            flush_norm()
                while pending:
                    flush_norm()
                return o

            def attention(qT, kT, vts, NQ, out_tag):
                """FM attention: O^T [DH, HEADS, NQ] bf16, softmax over S keys."""
                o = pfm.tile([DH, HEADS, NQ], bf16, tag=out_tag)
                pending = []

                def flush_norm():
                    if not pending:
                        return
                    po, h, n0, nn = pending.pop(0)
                    drow_bf = small.tile([1, 512], bf16, tag="drowbf")
                    nc.scalar.copy(out=drow_bf[:, 0:nn], in_=po[96:97, 0:nn])
                    prep = psA.tile([128, 512], f32, tag="psA")
                    nc.tensor.matmul(
                        out=prep[:, 0:nn], lhsT=ones_row[:],
                        rhs=drow_bf[:, 0:nn], start=True, stop=True)
                    recip_sb = pexp.tile([128, 512], f32, tag="recipsb", bufs=2)
                    nc.vector.reciprocal(out=recip_sb[0:DH, 0:nn], in_=prep[0:DH, 0:nn])
                    nc.vector.tensor_tensor(
                        out=o[:, h, n0:n0 + nn], in0=po[0:DH, 0:nn],
                        in1=recip_sb[0:DH, 0:nn], op=MUL)

                for h in range(HEADS):
                    for n0 in range(0, NQ, 512):
                        nn = min(512, NQ - n0)
                        po = psO.tile([128, 512], f32, tag="psO")
                        for tcid in range(NTC):
                            ps = psA.tile([128, 512], f32, tag="psA")
                            nc.tensor.matmul(
                                out=ps[:, 0:nn],
                                lhsT=kT[:, h, tcid * 128:(tcid + 1) * 128],
                                rhs=qT[:, h, n0:n0 + nn],
                                start=True, stop=True)
                            ex = pexp.tile([128, 512], bf16, tag="exp", bufs=3)
                            nc.scalar.activation(
                                out=ex[:, 0:nn], in_=ps[:, 0:nn],
                                func=mybir.ActivationFunctionType.Exp)
                            nc.tensor.matmul(
                                out=po[0:97, 0:nn],
                                lhsT=vts[tcid][:, h, 0:97],
                                rhs=ex[:, 0:nn],
                                start=(tcid == 0), stop=(tcid == NTC - 1))
                        pending.append((po, h, n0, nn))
                        if len(pending) > 1:
                            flush_norm()
                while pending:
                    flush_norm()
                return o

            # ================= COMP BRANCH (both frames first) =================
            ssum = [pfm.tile([128, D], f32, tag=f"ssum{sc}", name=f"ssum{sc}") for sc in range(NSC)]
            hs_tiles = []
            for f in range(FPC):
                hs_prev = load_hs(f, f"hsc{f}")
                hs_tiles.append(hs_prev)
                if f == 0:
                    g1, g2 = g1_pre, g2_pre
                else:
                    g1 = pfm.tile([128, NKC, K], bf16, tag="g1")
                    g2 = pfm.tile([128, NKC, K], bf16, tag="g2")
                    for kc in range(NKC):
                        nc.sync.dma_start(out=g1[:, kc, :], in_=g1t[f, kc * 128:(kc + 1) * 128, :])
                        nc.sync.dma_start(out=g2[:, kc, :], in_=g2t[f, kc * 128:(kc + 1) * 128, :])
                gd = pfm.tile([128, NKC, K], bf16, tag="zt")
                nc.vector.tensor_tensor(out=gd[:], in0=g1[:], in1=g2[:], op=SUB)

                # primary_hs (FM, chunk-aligned): prim = gd @ Wprim + b + g1
                prim = pfm.tile([128, NKC, K], bf16, tag="prim")
                for mc in range(NKC):
                    ps = psA.tile([128, 512], f32, tag="psA")
                    for kc in range(NKC):
                        nc.tensor.matmul(
                            out=ps[:], lhsT=wsb["wprim"][:, kc, mc * 128:(mc + 1) * 128],
                            rhs=gd[:, kc, :], start=(kc == 0), stop=(kc == NKC - 1))
                    nc.vector.tensor_scalar(
                        prim[:, mc, :], ps[:], bprim_sb[:, mc:mc + 1], None, op0=ADD)
                    nc.vector.tensor_tensor(
                        out=prim[:, mc, :], in0=prim[:, mc, :], in1=g1[:, mc, :], op=ADD)

                vC = proj_tm_v(hs_prev, wsb["wcv"], brepcv_sb, "v")
                ocT = fused_branch(wsb["wcq"], bcq_sb, wsb["wck"], bck_sb,
                                   prim, hs_prev, vC, K, "qT", "kT", "oT")

                # comp_out projection (TM) + bias; spill to DRAM + local sum
                for sc in range(NSC):
                    psL = psA.tile([128, 512], f32, tag="psA")
                    psR = psA.tile([128, 512], f32, tag="psA")
                    for h in range(HEADS):
                        nc.tensor.matmul(
                            out=psL[:, 0:512],
                            lhsT=ocT[:, h, sc * 128:(sc + 1) * 128],
                            rhs=wsb["wco"][:, h, 0:512],
                            start=(h == 0), stop=(h == HEADS - 1))
                    for h in range(HEADS):
                        nc.tensor.matmul(
                            out=psR[:, 0:128],
                            lhsT=ocT[:, h, sc * 128:(sc + 1) * 128],
                            rhs=wsb["wco"][:, h, 512:640],
                            start=(h == 0), stop=(h == HEADS - 1))
                    csb = small.tile([128, D], f32, tag="wk640")
                    nc.vector.tensor_tensor(out=csb[:, 0:512], in0=psL[:, 0:512],
                                            in1=brepco_sb[:, 0:512], op=ADD)
                    nc.vector.tensor_tensor(out=csb[:, 512:640], in0=psR[:, 0:128],
                                            in1=brepco_sb[:, 512:640], op=ADD)
                    nc.sync.dma_start(out=compd[f, sc * 128:(sc + 1) * 128, :], in_=csb[:])
                    if f == 0:
                        nc.vector.tensor_copy(ssum[sc][:], csb[:])
                    else:
                        nc.vector.tensor_tensor(out=ssum[sc][:], in0=ssum[sc][:], in1=csb[:], op=ADD)

            for sc in range(NSC):
                nc.sync.dma_start(out=cc_in[sc * 128:(sc + 1) * 128, :], in_=ssum[sc][:])
            nc.gpsimd.collective_compute(
                "AllGather", mybir.AluOpType.bypass,
                replica_groups=[list(range(NCORES))],
                ins=[cc_in.opt()], outs=[cc_out.opt()])

            # masked prefix, entirely on the gpsimd queue (inherently ordered
            # after the collective; keeps the PE/DVE/sync queues unblocked)
            pfx = [pfm.tile([128, D], f32, tag=f"ssum{sc}", name=f"pfx{sc}")
                   for sc in range(NSC)]
            for sc in range(NSC):
                nc.gpsimd.memset(pfx[sc][:], 0.0)
                for cc in range(NCORES):
                    tmp = small.tile([128, D], f32, tag="wk640b")
                    nc.gpsimd.dma_start(out=tmp[:], in_=cc_out[cc, sc * 128:(sc + 1) * 128, :])
                    nc.gpsimd.tensor_tensor(
                        out=tmp[:], in0=tmp[:],
                        in1=wmask_sb[:, cc:cc + 1].to_broadcast([128, D]), op=MUL)
                    nc.gpsimd.tensor_tensor(out=pfx[sc][:], in0=pfx[sc][:], in1=tmp[:], op=ADD)

            # ================= MAIN BRANCH =================
            for f in range(FPC):
                if f == 0:
                    hs_prev = hs_tiles[0]
                    hs_own = hs_tiles[1]
                else:
                    hs_prev = hs_tiles[1]
                    hs_own = load_hs(2, "hsm2")
                v2 = proj_tm_v(hs_prev, wsb["wv"], None, "v")
                o2T = fused_branch(wsb["wq"], None, wsb["wk"], None,
                                   hs_own, hs_prev, v2, S, "qT", "kT", "oT")

                # out projection -> Z^T bf16
                zt = pfm.tile([128, NKC, S], bf16, tag="zt")
                for mc in range(NKC):
                    for n0 in range(0, S, 512):
                        ps = psA.tile([128, 512], f32, tag="psA")
                        for h in range(HEADS):
                            nc.tensor.matmul(
                                out=ps[:],
                                lhsT=wsb["wo"][:, h, mc * 128:(mc + 1) * 128],
                                rhs=o2T[:, h, n0:n0 + 512],
                                start=(h == 0), stop=(h == HEADS - 1))
                        nc.scalar.copy(out=zt[:, mc, n0:n0 + 512], in_=ps[:])

                # transpose to TM, add bias, store
                for tcid in range(NTC):
                    pt = psT.tile([128, D], bf16, tag="psT")
                    for mc in range(NKC):
                        nc.tensor.transpose(
                            out=pt[:, mc * 128:(mc + 1) * 128],
                            in_=zt[:, mc, tcid * 128:(tcid + 1) * 128],
                            identity=ident[:])
                    zfin = small.tile([128, D], f32, tag="wk640")
                    nc.vector.tensor_tensor(out=zfin[:], in0=pt[:], in1=brepo_sb[:], op=ADD)
                    nc.sync.dma_start(out=out_ps[f][tcid * 128:(tcid + 1) * 128, :], in_=zfin[:])



            # cumsum + scatter-add RMW, entirely on the gpsimd queue;
            # phased per frame so the 4 indirect gathers pipeline their transfers
            for f in range(FPC):
                opv = out_ps[f][:]
                idxs = []
                for sc in range(NSC):
                    ctile = small.tile([128, D], f32, tag="wk640b")
                    nc.gpsimd.dma_start(out=ctile[:], in_=compd[f, sc * 128:(sc + 1) * 128, :])
                    nc.gpsimd.tensor_tensor(out=pfx[sc][:], in0=pfx[sc][:], in1=ctile[:], op=ADD)
                    idx = small.tile([128, 1], i32, tag="idx8", bufs=8)
                    nc.gpsimd.dma_start(out=idx[:], in_=topk[f, sc * 128:(sc + 1) * 128, None])
                    idxs.append(idx)
                for sc in range(NSC):
                    nc.gpsimd.indirect_dma_start(
                        out=opv, out_offset=bass.IndirectOffsetOnAxis(ap=idxs[sc][:, :1], axis=0),
                        in_=pfx[sc][:], in_offset=None,
                        compute_op=mybir.AluOpType.add)

    nc.compile()
    return nc


def _host_prep(hidden_states, topk_idx, weights):
    """Build the 8 per-core input maps (pure data movement + dtype casts)."""
    bf = ml_dtypes.bfloat16
    hs = np.asarray(hidden_states, np.float32)
    (wq, wk, wv, wo, wcq, wck, wcv, wco, wprim,
     b_prim, b_cq, b_ck, b_cv, b_co, b_o) = weights

    wq_s = (wq * SCALE).astype(bf)
    wcq_s = (wcq * SCALE).astype(bf)
    b_cq_s = (b_cq * SCALE).astype(np.float32)
    wcast = {
        "wq": wq_s, "wk": wk.astype(bf), "wv": wv.astype(bf), "wo": wo.astype(bf),
        "wcq": wcq_s, "wck": wck.astype(bf), "wcv": wcv.astype(bf),
        "wco": wco.astype(bf), "wprim": wprim.astype(bf),
    }
    shared = dict(wcast)
    shared["b_prim"] = b_prim.astype(np.float32)
    shared["b_cq"] = b_cq_s
    shared["b_ck"] = b_ck.astype(np.float32)
    shared["brep_cv"] = np.broadcast_to(b_cv.astype(bf), (128, D)).copy()
    shared["brep_co"] = np.broadcast_to(b_co.astype(bf), (128, D)).copy()
    shared["brep_o"] = np.broadcast_to(b_o.astype(bf), (128, D)).copy()

    in_maps = []
    for c in range(NCORES):
        f0, f1 = 2 * c, 2 * c + 1
        p0 = max(f0 - 1, 0)
        m = dict(shared)
        m["hsT"] = np.ascontiguousarray(
            hs[[p0, f0, f1]].transpose(0, 2, 1)).astype(bf)
        g1 = np.stack([hs[f0][topk_idx[f0]], hs[f1][topk_idx[f1]]])
        g2 = np.stack([hs[p0][topk_idx[f0]], hs[f0][topk_idx[f1]]])
        m["g1t"] = np.ascontiguousarray(g1.transpose(0, 2, 1)).astype(bf)
        m["g2t"] = np.ascontiguousarray(g2.transpose(0, 2, 1)).astype(bf)
        m["topk"] = np.ascontiguousarray(topk_idx[[f0, f1]]).astype(np.int32)
        wm = np.zeros((128, NCORES), np.float32)
        wm[:, :c] = 1.0
        m["wmask"] = wm
        in_maps.append(m)
    return in_maps


def kernel(hidden_states, primary_lin_w, primary_lin_b,
           comp_q_w, comp_q_b, comp_k_w, comp_k_b, comp_v_w, comp_v_b,
           comp_out_w, comp_out_b,
           to_q_w, to_k_w, to_v_w, to_out_w, to_out_b,
           video_length):
    import jax
    import jax.numpy as jnp
    from concourse.bass_utils import run_bass_kernel_spmd

    f = int(video_length)
    assert f == F and hidden_states.shape == (F, S, D)

    # Top-k selection: replicate the reference's exact eager-jax computation
    # (selection order is rounding-critical; must match bitwise).
    hs_j = jnp.asarray(np.asarray(hidden_states, np.float32))
    pre = jnp.concatenate([hs_j[:1], hs_j[:-1]], axis=0)
    diff_score = jnp.abs(hs_j - pre).mean(axis=-1)
    _, topk_idx = jax.lax.top_k(diff_score, max(64, S // 2))
    topk_idx = np.asarray(topk_idx)

    weights = tuple(np.asarray(w, np.float32) for w in (
        to_q_w, to_k_w, to_v_w, to_out_w, comp_q_w, comp_k_w, comp_v_w,
        comp_out_w, primary_lin_w, primary_lin_b, comp_q_b, comp_k_b,
        comp_v_b, comp_out_b, to_out_b))
    in_maps = _host_prep(hidden_states, topk_idx, weights)

    if "nc" not in _cache:
        _cache["nc"] = _build_nc()
    res = run_bass_kernel_spmd(_cache["nc"], in_maps, list(range(NCORES)))
    out = np.concatenate(
        [np.stack([res.results[c]["out0"], res.results[c]["out1"]])
         for c in range(NCORES)], axis=0)
    return np.ascontiguousarray(out.astype(np.float32))
